# revision 1
# baseline (speedup 1.0000x reference)
"""Trainium2 Bass kernel for nn_DCGAN_G (DCGAN generator + 69-neuron spiking scan).

Strategy (8 NeuronCores, SPMD):
  A. W_in matvec (12800x2048) row-sharded 8x -> AllGather h1 (12800).
  B. DCGAN conv stack replicated on every core (tiny: ~3 GMAC).
  C. W_d2 matvec (4761x6400) row-sharded 8x -> AllGather w (69x69).
  D. 99800-step spiking recurrence (inherently serial): faithful
     per-step {matmul, tanh, subtract} chain; states collected as
     (69, T) columns, then PE-transposed to the (T, 69) output.
"""
import numpy as np

import bass_rust
import concourse.bass as bass
import concourse.mybir as mybir
from concourse.bass_utils import run_bass_kernel_spmd
from concourse.tile import TileContext
from concourse.vector_clock import ScopedClock

f32 = mybir.dt.float32
AF = mybir.ActivationFunctionType
OP = mybir.AluOpType
AX = mybir.AxisListType

T_FULL = 99800
N = 69
NCORES = 8
EPS = 1e-5
SCAN_B = 499          # 499 * 200 == 99800 exactly
MROWS_A = 1600        # W_in rows per core
MROWS_C = 596         # W_d2 rows per core (8*596=4768 >= 4761)


# ---------------------------------------------------------------------------
# walrus workaround: CTRL-type instructions accept at most 1 sem wait, but the
# TileContext tail drain gets one wait per active proc. Split across drains.
def _patched_drain_and_barrier(self, tick_clock, wait_clock):
    drain_inst = self.nc.sync.drain()
    wait_clock.add_sem_waits(
        drain_inst.ins, ScopedClock({None: tick_clock.global_clock})
    )
    si = drain_inst.ins.sync_info
    waits = list(si.on_wait) if si is not None else []
    if len(waits) > 1:
        drain_inst.ins.sync_info = bass_rust.SyncInfo(
            on_wait=waits[:1], on_update=list(si.on_update)
        )
        for i in range(1, len(waits)):
            extra = self.nc.sync.drain()
            extra.ins.sync_info = bass_rust.SyncInfo(
                on_wait=waits[i : i + 1], on_update=[]
            )
    self.nc.all_engine_barrier()
    assert self.sems is not None
    popped = self.nc._tile_sem_poison_stack.pop()
    assert popped is self._sem_poison
    self.nc.clear_and_free_semaphores(list(self.sems.allocated().values()))
    self.nc.all_engine_barrier()


TileContext._drain_and_barrier = _patched_drain_and_barrier
# ---------------------------------------------------------------------------


def _split_excess_waits(nc, max_waits=1):
    """This walrus build accepts at most one sem wait per instruction; move
    excess waits onto single-wait NOPs inserted just before the owner."""
    n_split = 0
    for f in nc.m.functions:
        for b in f.blocks:
            insts = list(b.instructions)
            out = []
            changed = False
            for inst in insts:
                si = inst.sync_info
                waits = list(si.on_wait) if si is not None else []
                if len(waits) > max_waits:
                    changed = True
                    for i, w in enumerate(waits[max_waits:]):
                        nop = mybir.InstNoOp(
                            name=f"wsp_{inst.name}_{i}", ins=[], outs=[])
                        nop.engine = inst.engine
                        nop.sync_info = bass_rust.SyncInfo(
                            on_wait=[w], on_update=[])
                        out.append(nop)
                        n_split += 1
                    inst.sync_info = bass_rust.SyncInfo(
                        on_wait=waits[:max_waits], on_update=list(si.on_update))
                out.append(inst)
            if changed:
                b.instructions = out
    return n_split


def _pad_w5(w5):
    """(1,64,4,4) -> (4,4,64,32) with real weights in out-column 0."""
    t = np.zeros((4, 4, 64, 32), np.float32)
    t[:, :, :, 0:1] = w5.transpose(2, 3, 1, 0)
    return np.ascontiguousarray(t)


def _col_major_pad(v, ncols):
    """(n,) -> (128, ncols) with element m at [m % 128, m // 128], zero pad."""
    out = np.zeros(128 * ncols, np.float32)
    out[: v.shape[0]] = v
    return np.ascontiguousarray(out.reshape(ncols, 128).T)


def build_program(T=T_FULL, with_front=True, with_scan=True):
    nc = bass.Bass()
    nblk = (T + SCAN_B - 1) // SCAN_B
    assert nblk * SCAN_B == T, "T must be a multiple of SCAN_B"

    # ---- inputs ----
    x_cols = nc.declare_dram_parameter("x_cols", [128, 16], f32, isOutput=False)
    win_t = nc.declare_dram_parameter("win_t", [2048, MROWS_A], f32, isOutput=False)
    bin_c = nc.declare_dram_parameter("bin_c", [128, 13], f32, isOutput=False)
    w1t = nc.declare_dram_parameter("w1t", [4, 4, 512, 512], f32, isOutput=False)
    w2t = nc.declare_dram_parameter("w2t", [4, 4, 512, 256], f32, isOutput=False)
    w3t = nc.declare_dram_parameter("w3t", [4, 4, 256, 128], f32, isOutput=False)
    w4t = nc.declare_dram_parameter("w4t", [4, 4, 128, 64], f32, isOutput=False)
    w5t = nc.declare_dram_parameter("w5t", [4, 4, 64, 32], f32, isOutput=False)
    g_all = nc.declare_dram_parameter("g_all", [128, 8], f32, isOutput=False)
    be_all = nc.declare_dram_parameter("be_all", [128, 8], f32, isOutput=False)
    wd2_t = nc.declare_dram_parameter("wd2_t", [6400, MROWS_C], f32, isOutput=False)
    bd2_c = nc.declare_dram_parameter("bd2_c", [128, 5], f32, isOutput=False)
    s0_in = nc.declare_dram_parameter("s0", [N, 1], f32, isOutput=False)
    ident_in = nc.declare_dram_parameter("ident", [128, 128], f32, isOutput=False)
    if with_scan:
        out_traj = nc.declare_dram_parameter("out", [T, N], f32, isOutput=True)
    else:
        w_out = nc.declare_dram_parameter("w_out", [N, N], f32, isOutput=True)

    # ---- internal DRAM ----
    h_shard = nc.dram_tensor("h_shard", [MROWS_A], f32)
    h_full = nc.dram_tensor("h_full", [NCORES * MROWS_A], f32, addr_space="Shared")
    c_scr = nc.dram_tensor("c_scr", [32, 6400], f32)
    wd_shard = nc.dram_tensor("wd_shard", [MROWS_C], f32)
    w_full = nc.dram_tensor("w_full", [NCORES * MROWS_C], f32, addr_space="Shared")
    traj = nc.dram_tensor("traj", [N, T], f32)

    with TileContext(nc) as tc:
        # ================= Phase A: h = W_in @ x + b_in (sharded) ==========
        with (
            tc.tile_pool(name="a_const", bufs=1) as acp,
            tc.tile_pool(name="a_slab", bufs=2) as asp,
            tc.tile_pool(name="a_ps", bufs=1, space="PSUM") as aps,
        ):
            xc = acp.tile([128, 16], f32)
            nc.sync.dma_start(out=xc[:, :], in_=x_cols[:, :])
            bc = acp.tile([128, 13], f32)
            nc.sync.dma_start(out=bc[:, :], in_=bin_c[:, :])
            hc = acp.tile([128, 13], f32)
            for jlo, jhi in ((0, 8), (8, 13)):
                ptiles = {}
                for j in range(jlo, jhi):
                    pt = aps.tile([128, 1], f32, tag=f"hps{j - jlo}", name=f"hps{j}")
                    ptiles[j] = pt
                for k in range(16):
                    gw = min(128 * jhi, MROWS_A) - 128 * jlo
                    slab = asp.tile([128, 1024], f32, tag="aslab")
                    nc.sync.dma_start(
                        out=slab[:, :gw],
                        in_=win_t[128 * k : 128 * (k + 1),
                                  128 * jlo : 128 * jlo + gw])
                    for j in range(jlo, jhi):
                        cj = 128 if j < 12 else 64
                        jj = j - jlo
                        nc.tensor.matmul(
                            ptiles[j][:cj, :],
                            slab[:, 128 * jj : 128 * jj + cj],
                            xc[:, k : k + 1],
                            start=(k == 0),
                            stop=(k == 15),
                        )
                for j in range(jlo, jhi):
                    cj = 128 if j < 12 else 64
                    nc.vector.tensor_tensor(
                        out=hc[:cj, j : j + 1], in0=ptiles[j][:cj, :],
                        in1=bc[:cj, j : j + 1], op=OP.add)
            for j in range(13):
                cj = 128 if j < 12 else 64
                nc.sync.dma_start(
                    out=h_shard[128 * j : 128 * j + cj], in_=hc[:cj, j])
        nc.gpsimd.collective_compute(
            "AllGather", OP.bypass, replica_groups=[list(range(NCORES))],
            ins=[h_shard[:]], outs=[h_full[:]])

        # ================= Phase B: conv stack (replicated) ================
        _lvl = 9  # all conv layers (bisection gates left in place, fully on)
        h2d = h_full.rearrange("(c hw) -> c hw", hw=25)
        gsl = {1: (0, 4), 2: (4, 2), 3: (6, 1), 4: (7, 1)}  # (col offset, ncols)

        with (
            tc.tile_pool(name="bn_const", bufs=1) as bnp,
            tc.tile_pool(name="conv_ps", bufs=1, space="PSUM") as bps,
        ):
            g_sb = bnp.tile([128, 8], f32)
            nc.sync.dma_start(out=g_sb[:, :], in_=g_all[:, :])
            be_sb = bnp.tile([128, 8], f32)
            nc.sync.dma_start(out=be_sb[:, :], in_=be_all[:, :])

            def bn_relu(raw, hw, cch, lidx, j, out_ap):
                """BatchNorm(train) + ReLU from raw (cch,hw) into out_ap."""
                with tc.tile_pool(name=f"bn{lidx}_{j}", bufs=1) as p:
                    s1 = p.tile([cch, 1], f32, tag="s1")
                    nc.vector.tensor_reduce(s1[:, :], raw, axis=AX.X, op=OP.add)
                    mean = p.tile([cch, 1], f32, tag="mean")
                    nc.vector.tensor_scalar_mul(mean[:, :], s1[:, :], 1.0 / hw)
                    sq = p.tile([cch, hw], f32, tag="sq")
                    nc.vector.tensor_tensor(out=sq[:, :], in0=raw, in1=raw, op=OP.mult)
                    s2 = p.tile([cch, 1], f32, tag="s2")
                    nc.vector.tensor_reduce(s2[:, :], sq[:, :], axis=AX.X, op=OP.add)
                    ex2 = p.tile([cch, 1], f32, tag="ex2")
                    nc.vector.tensor_scalar_mul(ex2[:, :], s2[:, :], 1.0 / hw)
                    msq = p.tile([cch, 1], f32, tag="msq")
                    nc.vector.tensor_tensor(
                        out=msq[:, :], in0=mean[:, :], in1=mean[:, :], op=OP.mult)
                    var = p.tile([cch, 1], f32, tag="var")
                    nc.vector.tensor_tensor(
                        out=var[:, :], in0=ex2[:, :], in1=msq[:, :], op=OP.subtract)
                    vps = p.tile([cch, 1], f32, tag="vps")
                    nc.vector.tensor_scalar_add(vps[:, :], var[:, :], EPS)
                    sd = p.tile([cch, 1], f32, tag="sd")
                    nc.scalar.activation(sd[:, :], vps[:, :], AF.Sqrt)
                    rstd = p.tile([cch, 1], f32, tag="rstd")
                    nc.vector.reciprocal(rstd[:, :], sd[:, :])
                    co, _ = gsl[lidx]
                    scale = p.tile([cch, 1], f32, tag="scale")
                    nc.vector.tensor_tensor(
                        out=scale[:, :], in0=g_sb[:cch, co + j : co + j + 1],
                        in1=rstd[:, :], op=OP.mult)
                    t1 = p.tile([cch, 1], f32, tag="t1")
                    nc.vector.tensor_tensor(
                        out=t1[:, :], in0=mean[:, :], in1=scale[:, :], op=OP.mult)
                    bia = p.tile([cch, 1], f32, tag="bia")
                    nc.vector.tensor_tensor(
                        out=bia[:, :], in0=be_sb[:cch, co + j : co + j + 1],
                        in1=t1[:, :], op=OP.subtract)
                    nc.scalar.activation(
                        out_ap, raw, AF.Relu, bias=bia[:, :], scale=scale[:, :])

            # ---- L1: up2(h:512x5x5)->512x10x10 conv 512->512 ----
            with (
                tc.tile_pool(name="l1_in", bufs=1) as l1i,
                tc.tile_pool(name="l1_w", bufs=2) as l1w,
                tc.tile_pool(name="l1_out", bufs=1) as l1o,
            ):
                pads1 = []
                for j in range(4):
                    hm = l1i.tile([128, 25], f32, tag=f"hm{j}")
                    nc.sync.dma_start(out=hm[:, :], in_=h2d[128 * j : 128 * (j + 1), :])
                    pad = l1i.tile([128, 13 * 13], f32, tag=f"pad1_{j}")
                    nc.vector.memset(pad[:, :], 0.0)
                    pv = pad[:, :].rearrange("c (h w) -> c h w", h=13)
                    hv = hm[:, :].rearrange("c (h w) -> c h w", h=5)
                    for a in range(2):
                        for b in range(2):
                            nc.vector.tensor_copy(
                                pv[:, a + 1 : a + 11 : 2, b + 1 : b + 11 : 2], hv[:, :, :])
                    pads1.append(pad)
                ps1s = []
                for jo in range(4):
                    p1 = bps.tile([128, 100], f32, tag=f"l1ps{jo}", name=f"l1ps{jo}")
                    ps1s.append(p1)
                nmm = 0
                for ji in range(4):
                    for dy in range(4):
                        for dx in range(4):
                            slab = l1w.tile([128, 512], f32, tag="w1slab")
                            nc.sync.dma_start(
                                out=slab[:, :],
                                in_=w1t[dy, dx, 128 * ji : 128 * (ji + 1), :])
                            rhs = pads1[ji][:, :].rearrange(
                                "c (h w) -> c h w", h=13)[:, dy : dy + 10, dx : dx + 10]
                            for jo in range(4):
                                nc.tensor.matmul(
                                    ps1s[jo][:, :],
                                    slab[:, 128 * jo : 128 * (jo + 1)], rhs,
                                    start=(nmm == 0), stop=(nmm == 63))
                            nmm += 1
                pads2 = []
                for jo in range(4):
                    raw = l1o.tile([128, 100], f32, tag=f"raw1_{jo}")
                    nc.vector.tensor_copy(raw[:, :], ps1s[jo][:, :])
                    relu = l1o.tile([128, 100], f32, tag=f"relu1_{jo}")
                    bn_relu(raw[:, :], 100, 128, 1, jo, relu[:, :])
                    pad = l1o.tile([128, 23 * 23], f32, tag=f"pad2_{jo}")
                    nc.vector.memset(pad[:, :], 0.0)
                    pv = pad[:, :].rearrange("c (h w) -> c h w", h=23)
                    rv = relu[:, :].rearrange("c (h w) -> c h w", h=10)
                    for a in range(2):
                        for b in range(2):
                            nc.vector.tensor_copy(
                                pv[:, a + 1 : a + 21 : 2, b + 1 : b + 21 : 2], rv[:, :, :])
                    pads2.append(pad)

                if _lvl >= 2:
                  # ---- L2: 512x20x20 conv 512->256 ----
                  with (
                      tc.tile_pool(name="l2_w", bufs=2) as l2w,
                      tc.tile_pool(name="l2_out", bufs=1) as l2o,
                  ):
                      psA = bps.tile([128, 400], f32, tag="cpsA")
                      psB = bps.tile([128, 400], f32, tag="cpsB")
                      nmm = 0
                      for ji in range(4):
                          for dy in range(4):
                              for dx in range(4):
                                  slab = l2w.tile([128, 256], f32, tag="w2slab")
                                  nc.sync.dma_start(
                                      out=slab[:, :],
                                      in_=w2t[dy, dx, 128 * ji : 128 * (ji + 1), :])
                                  rhs = pads2[ji][:, :].rearrange(
                                      "c (h w) -> c h w", h=23)[:, dy : dy + 20, dx : dx + 20]
                                  nc.tensor.matmul(
                                      psA[:, :], slab[:, 0:128], rhs,
                                      start=(nmm == 0), stop=(nmm == 63))
                                  nc.tensor.matmul(
                                      psB[:, :], slab[:, 128:256], rhs,
                                      start=(nmm == 0), stop=(nmm == 63))
                                  nmm += 1
                      pads3 = []
                      for jo, ps in enumerate((psA, psB)):
                          raw = l2o.tile([128, 400], f32, tag=f"raw2_{jo}")
                          nc.vector.tensor_copy(raw[:, :], ps[:, :])
                          relu = l2o.tile([128, 400], f32, tag=f"relu2_{jo}")
                          bn_relu(raw[:, :], 400, 128, 2, jo, relu[:, :])
                          pad = l2o.tile([128, 43 * 43], f32, tag=f"pad3_{jo}")
                          nc.vector.memset(pad[:, :], 0.0)
                          pv = pad[:, :].rearrange("c (h w) -> c h w", h=43)
                          rv = relu[:, :].rearrange("c (h w) -> c h w", h=20)
                          for a in range(2):
                              for b in range(2):
                                  nc.vector.tensor_copy(
                                      pv[:, a + 1 : a + 41 : 2, b + 1 : b + 41 : 2],
                                      rv[:, :, :])
                          pads3.append(pad)

                      if _lvl >= 3:
                        # ---- L3: 256x40x40 conv 256->128 ----
                        with (
                            tc.tile_pool(name="l3_w", bufs=1) as l3w,
                            tc.tile_pool(name="l3_out", bufs=1) as l3o,
                        ):
                            wsl3 = l3w.tile([128, 32 * 128], f32)
                            for ji in range(2):
                                for dy in range(4):
                                    for dx in range(4):
                                        si = (ji * 16 + dy * 4 + dx) * 128
                                        nc.sync.dma_start(
                                            out=wsl3[:, si : si + 128],
                                            in_=w3t[dy, dx, 128 * ji : 128 * (ji + 1), :])
                            raw3 = l3o.tile([128, 1600], f32)
                            for st in range(4):
                                ps = bps.tile([128, 400], f32, tag="cps", bufs=2)
                                nmm = 0
                                for ji in range(2):
                                    for dy in range(4):
                                        for dx in range(4):
                                            si = (ji * 16 + dy * 4 + dx) * 128
                                            rhs = pads3[ji][:, :].rearrange(
                                                "c (h w) -> c h w", h=43)[
                                                :, st * 10 + dy : st * 10 + dy + 10,
                                                dx : dx + 40]
                                            nc.tensor.matmul(
                                                ps[:, :], wsl3[:, si : si + 128], rhs,
                                                start=(nmm == 0), stop=(nmm == 31))
                                            nmm += 1
                                nc.vector.tensor_copy(
                                    raw3[:, 400 * st : 400 * (st + 1)], ps[:, :])
                            relu3 = l3o.tile([128, 1600], f32)
                            bn_relu(raw3[:, :], 1600, 128, 3, 0, relu3[:, :])
                            pad4 = l3o.tile([128, 83 * 83], f32)
                            nc.vector.memset(pad4[:, :], 0.0)
                            pv = pad4[:, :].rearrange("c (h w) -> c h w", h=83)
                            rv = relu3[:, :].rearrange("c (h w) -> c h w", h=40)
                            for a in range(2):
                                for b in range(2):
                                    nc.vector.tensor_copy(
                                        pv[:, a + 1 : a + 81 : 2, b + 1 : b + 81 : 2],
                                        rv[:, :, :])

                            if _lvl >= 4:
                              # ---- L4: 128x80x80 conv 128->64 ----
                              with (
                                  tc.tile_pool(name="l4_w", bufs=1) as l4w,
                                  tc.tile_pool(name="l4_out", bufs=1) as l4o,
                              ):
                                  wsl4 = l4w.tile([128, 16 * 64], f32)
                                  for dy in range(4):
                                      for dx in range(4):
                                          si = (dy * 4 + dx) * 64
                                          nc.sync.dma_start(
                                              out=wsl4[:, si : si + 64],
                                              in_=w4t[dy, dx, :, :])
                                  raw4 = l4o.tile([64, 6400], f32)
                                  for st in range(16):
                                      ps = bps.tile([64, 400], f32, tag="cps", bufs=2)
                                      nmm = 0
                                      for dy in range(4):
                                          for dx in range(4):
                                              si = (dy * 4 + dx) * 64
                                              rhs = pad4[:, :].rearrange(
                                                  "c (h w) -> c h w", h=83)[
                                                  :, st * 5 + dy : st * 5 + dy + 5,
                                                  dx : dx + 80]
                                              nc.tensor.matmul(
                                                  ps[:, :], wsl4[:, si : si + 64], rhs,
                                                  start=(nmm == 0), stop=(nmm == 15))
                                              nmm += 1
                                      nc.vector.tensor_copy(
                                          raw4[:, 400 * st : 400 * (st + 1)], ps[:, :])
                                  pad5 = l4o.tile([64, 83 * 83], f32)
                                  nc.vector.memset(pad5[:, :], 0.0)
                                  pv5 = pad5[:, :].rearrange("c (h w) -> c h w", h=83)[
                                      :, 1:81, 1:81]
                                  bn_relu(raw4[:, :], 6400, 64, 4, 0, pv5)

                                  if _lvl >= 5:
                                    # ---- L5: 64x80x80 conv 64->1 + tanh -> c ----
                                    with (
                                        tc.tile_pool(name="l5_w", bufs=1) as l5w,
                                        tc.tile_pool(name="l5_out", bufs=1) as l5o,
                                    ):
                                        wsl5 = l5w.tile([64, 16 * 32], f32)
                                        for dy in range(4):
                                            for dx in range(4):
                                                _p5 = (dy * 4 + dx) * 32
                                                nc.sync.dma_start(
                                                    out=wsl5[:, _p5 : _p5 + 32],
                                                    in_=w5t[dy, dx, :, :])
                                        for st in range(16):
                                            ps = bps.tile([32, 400], f32, tag="cps", bufs=2)
                                            nmm = 0
                                            for dy in range(4):
                                                for dx in range(4):
                                                    rhs = pad5[:, :].rearrange(
                                                        "c (h w) -> c h w", h=83)[
                                                        :, st * 5 + dy : st * 5 + dy + 5,
                                                        dx : dx + 80]
                                                    _p5 = (dy * 4 + dx) * 32
                                                    nc.tensor.matmul(
                                                        ps[:, :],
                                                        wsl5[:, _p5 : _p5 + 32],
                                                        rhs,
                                                        start=(nmm == 0), stop=(nmm == 15))
                                                    nmm += 1
                                            c32 = l5o.tile([32, 400], f32, tag="c32", name=f"c32_{st}")
                                            nc.scalar.activation(c32[:, :], ps[:, :], AF.Tanh)
                                            nc.sync.dma_start(
                                                out=c_scr[:, 400 * st : 400 * (st + 1)], in_=c32[:, :])

        # ================= Phase C: w = W_d2 @ c + b_d2 (sharded) ==========
        _skip_c = False
        if not _skip_c:
          with (
              tc.tile_pool(name="c_const", bufs=1) as ccp,
              tc.tile_pool(name="c_slab", bufs=2) as csp,
              tc.tile_pool(name="c_ps", bufs=1, space="PSUM") as cps,
          ):
              c_cols = ccp.tile([128, 50], f32)
              nc.sync.dma_start(
                  out=c_cols[:, :], in_=c_scr[0, :].rearrange("(f p) -> p f", p=128))
              bdc = ccp.tile([128, 5], f32)
              nc.sync.dma_start(out=bdc[:, :], in_=bd2_c[:, :])
              wtiles = {}
              for j in range(5):
                  wt_ps = cps.tile([128, 1], f32, tag=f"wps{j}", name=f"wps{j}")
                  wtiles[j] = wt_ps
              for k in range(50):
                  slab = csp.tile([128, MROWS_C], f32, tag="cslab")
                  nc.sync.dma_start(
                      out=slab[:, :], in_=wd2_t[128 * k : 128 * (k + 1), :])
                  for j in range(5):
                      cj = 128 if j < 4 else 84
                      nc.tensor.matmul(
                          wtiles[j][:cj, :], slab[:, 128 * j : 128 * j + cj],
                          c_cols[:, k : k + 1], start=(k == 0), stop=(k == 49))
              wdc = ccp.tile([128, 5], f32)
              for j in range(5):
                  cj = 128 if j < 4 else 84
                  nc.vector.tensor_tensor(
                      out=wdc[:cj, j : j + 1], in0=wtiles[j][:cj, :],
                      in1=bdc[:cj, j : j + 1], op=OP.add)
              for j in range(5):
                  cj = 128 if j < 4 else 84
                  nc.sync.dma_start(
                      out=wd_shard[128 * j : 128 * j + cj], in_=wdc[:cj, j])
        if not _skip_c:
            nc.gpsimd.collective_compute(
                "AllGather", OP.bypass, replica_groups=[list(range(NCORES))],
                ins=[wd_shard[:]], outs=[w_full[:]])

        if not with_scan:
            with tc.tile_pool(name="wout", bufs=1) as wop:
                w_sb0 = wop.tile([N, N], f32)
                nc.sync.dma_start(
                    out=w_sb0[:, :],
                    in_=w_full[0 : N * N].rearrange("(j i) -> j i", i=N))
                nc.sync.dma_start(out=w_out[:, :], in_=w_sb0[:, :])

        # ================= Phase D: spiking scan =========================
        if with_scan:
          with (
              tc.tile_pool(name="d_const", bufs=1) as dcp,
              tc.tile_pool(name="d_state", bufs=1) as dsp,
              tc.tile_pool(name="d_blk", bufs=3) as dbp,
              tc.tile_pool(name="d_ps", bufs=2, space="PSUM") as dps,
          ):
              w_sb = dcp.tile([N, N], f32)
              nc.sync.dma_start(
                  out=w_sb[:, :],
                  in_=w_full[0 : N * N].rearrange("(j i) -> j i", i=N))
              s_col = dsp.tile([N, 1], f32)
              nc.sync.dma_start(out=s_col[:, :], in_=s0_in[:, :])
              u_col = dsp.tile([N, 1], f32)
              with tc.For_i(
                  0, T, SCAN_B,
                  hint_engines=(
                      mybir.EngineType.PE, mybir.EngineType.Activation,
                      mybir.EngineType.DVE),
              ) as iv:
                  sblk = dbp.tile([N, SCAN_B], f32, tag="sblk")
                  for k in range(SCAN_B):
                      y_ps = dps.tile([N, 1], f32, tag="y")
                      prev = s_col[:, :] if k == 0 else sblk[:, k - 1 : k]
                      nc.tensor.matmul(
                          y_ps[:, :], w_sb[:, :], prev, start=True, stop=True)
                      nc.scalar.activation(u_col[:, :], y_ps[:, :], AF.Tanh)
                      nc.vector.tensor_tensor(
                          out=sblk[:, k : k + 1], in0=u_col[:, :], in1=prev,
                          op=OP.subtract)
                  nc.vector.tensor_copy(s_col[:, :], sblk[:, SCAN_B - 1 : SCAN_B])
                  nc.sync.dma_start(out=traj[:, bass.ds(iv, SCAN_B)], in_=sblk[:, :])

              # ---- transpose (69, T) -> (T, 69) ----
              with (
                  tc.tile_pool(name="t_in", bufs=3) as tip,
                  tc.tile_pool(name="t_ps", bufs=2, space="PSUM") as tpp,
              ):
                  ident = dcp.tile([128, 128], f32)
                  nc.sync.dma_start(out=ident[:, :], in_=ident_in[:, :])

                  def transpose_tile(col_expr, width):
                      tin = tip.tile([N, 128], f32, tag="tin")
                      nc.sync.dma_start(
                          out=tin[:, :width], in_=traj[:, bass.ds(col_expr, width)])
                      tps = tpp.tile([128, N], f32, tag="tps")
                      nc.tensor.transpose(
                          tps[:width, :], tin[:, :width], ident[:N, :N])
                      tsb = tip.tile([128, N], f32, tag="tsb")
                      nc.vector.tensor_copy(tsb[:width, :], tps[:width, :])
                      nc.sync.dma_start(
                          out=out_traj[bass.ds(col_expr, width), :],
                          in_=tsb[:width, :])

                  col = 0
                  while col < T:
                      wdt = min(128, T - col)
                      transpose_tile(col, wdt)
                      col += wdt

    return nc


def _marshal_inputs(inputs):
    """Build the 8 per-core input maps from the full problem inputs."""
    x = np.asarray(inputs["x"], np.float32).reshape(2048)
    win = np.asarray(inputs["W_in"], np.float32)
    b_in = np.asarray(inputs["b_in"], np.float32)
    wd2 = np.asarray(inputs["W_d2"], np.float32)
    bd2 = np.asarray(inputs["b_d2"], np.float32)
    sp = np.asarray(inputs["start_part"], np.float32)

    x_cols = np.ascontiguousarray(x.reshape(16, 128).T)
    g_all = np.zeros((128, 8), np.float32)
    be_all = np.zeros((128, 8), np.float32)
    g_all[:, 0:4] = _col_major_pad(np.asarray(inputs["g1"], np.float32), 4)
    g_all[:, 4:6] = _col_major_pad(np.asarray(inputs["g2"], np.float32), 2)
    g_all[:, 6:7] = _col_major_pad(np.asarray(inputs["g3"], np.float32), 1)
    g_all[:, 7:8] = _col_major_pad(np.asarray(inputs["g4"], np.float32), 1)
    be_all[:, 0:4] = _col_major_pad(np.asarray(inputs["be1"], np.float32), 4)
    be_all[:, 4:6] = _col_major_pad(np.asarray(inputs["be2"], np.float32), 2)
    be_all[:, 6:7] = _col_major_pad(np.asarray(inputs["be3"], np.float32), 1)
    be_all[:, 7:8] = _col_major_pad(np.asarray(inputs["be4"], np.float32), 1)
    wts = {
        "w1t": np.ascontiguousarray(
            np.asarray(inputs["w1"], np.float32).transpose(2, 3, 1, 0)),
        "w2t": np.ascontiguousarray(
            np.asarray(inputs["w2"], np.float32).transpose(2, 3, 1, 0)),
        "w3t": np.ascontiguousarray(
            np.asarray(inputs["w3"], np.float32).transpose(2, 3, 1, 0)),
        "w4t": np.ascontiguousarray(
            np.asarray(inputs["w4"], np.float32).transpose(2, 3, 1, 0)),
        "w5t": _pad_w5(np.asarray(inputs["w5"], np.float32)),
    }
    s0 = np.ascontiguousarray(sp[-1].reshape(N, 1))
    ident = np.eye(128, dtype=np.float32)

    wd2_pad = np.zeros((NCORES * MROWS_C, 6400), np.float32)
    wd2_pad[: wd2.shape[0]] = wd2
    bd2_pad = np.zeros(NCORES * MROWS_C, np.float32)
    bd2_pad[: bd2.shape[0]] = bd2

    in_maps = []
    for c in range(NCORES):
        m = {
            "x_cols": x_cols,
            "win_t": np.ascontiguousarray(
                win[MROWS_A * c : MROWS_A * (c + 1)].T),
            "bin_c": _col_major_pad(b_in[MROWS_A * c : MROWS_A * (c + 1)], 13),
            "g_all": g_all,
            "be_all": be_all,
            "wd2_t": np.ascontiguousarray(
                wd2_pad[MROWS_C * c : MROWS_C * (c + 1)].T),
            "bd2_c": _col_major_pad(bd2_pad[MROWS_C * c : MROWS_C * (c + 1)], 5),
            "s0": s0,
            "ident": ident,
        }
        m.update(wts)
        in_maps.append(m)
    return in_maps


LAST_EXEC_NS = None


def kernel(**inputs) -> np.ndarray:
    global LAST_EXEC_NS
    import os

    trace = bool(os.environ.get("KERNEL_TRACE"))
    nc = build_program(T_FULL)
    _split_excess_waits(nc)
    in_maps = _marshal_inputs(inputs)
    res = run_bass_kernel_spmd(nc, in_maps, list(range(NCORES)), trace=trace)
    if res.exec_time_ns is not None:
        LAST_EXEC_NS = res.exec_time_ns
    out = np.asarray(res.results[0]["out"], np.float32)
    return out.reshape(1, T_FULL, N)


if __name__ == "__main__":
    # CoreSim selftest with a short scan (no hardware needed).
    import sys
    import time

    T_test = SCAN_B * 2
    nc = build_program(T_test)
    print("program built", flush=True)

    sys.path.insert(0, "/root/problem")
    import jax
    jax.config.update("jax_platform_name", "cpu")
    import reference

    inputs = reference.setup_inputs()
    inputs = {k: np.asarray(v) for k, v in inputs.items()}
    in_maps = _marshal_inputs(inputs)

    from concourse.bass_interp import MultiCoreSim

    t0 = time.time()
    sim = MultiCoreSim(nc, NCORES)
    for i in range(NCORES):
        for k, v in in_maps[i].items():
            sim.cores[i].tensor(k)[:] = v
    sim.simulate()
    print("sim time", time.time() - t0, flush=True)
    got = np.array(sim.cores[0].tensor("out"))

    # host reference for the short horizon
    w = np.load("/tmp/w.npy")
    s = np.asarray(inputs["start_part"])[-1].astype(np.float32)
    ref = np.empty((T_test, N), np.float32)
    for t in range(T_test):
        s = (np.tanh((s @ w).astype(np.float32)).astype(np.float32) - s).astype(
            np.float32)
        ref[t] = s
    err = np.abs(got - ref)
    rel = np.abs(got - ref) / (np.abs(ref) + 1e-6)
    print("traj absmax err:", err.max(), "rel max:", rel.max())
    print("first rows got:", got[0, :4], "ref:", ref[0, :4])



# revision 6
# speedup vs baseline: 1.1775x; 1.1775x over previous
"""Trainium2 Bass kernel for nn_DCGAN_G (DCGAN generator + 69-neuron spiking scan).

Strategy (8 NeuronCores, SPMD):
  A. W_in matvec (12800x2048) row-sharded 8x -> AllGather h1 (12800).
  B. DCGAN conv stack replicated on every core (tiny: ~3 GMAC).
  C. W_d2 matvec (4761x6400) row-sharded 8x -> AllGather w (69x69).
  D. 99800-step spiking recurrence (inherently serial).  Critical cycle is
     tanh -> matmul -> tanh only: with u_t = tanh(y_t) the next matvec is
     computed as y_{t+1} = w.u_t - w.s_t (two accumulating matmuls; the
     second operand s_t is a full step old), so the elementwise subtract
     s_{t+1} = u_t - s_t runs on DVE OFF the critical path.  Sem-wait
     hygiene keeps the one critical wait on each instruction (engine-level
     wait) and NoOp-splits only early-satisfied waits.  Output rows are
     PE-transposed in 128-column chunks inside the loop (PE is otherwise
     idle), so no separate transpose phase and no DRAM round-trip.
"""
import numpy as np

import bass_rust
import concourse.bass as bass
import concourse.mybir as mybir
from concourse.bass_utils import run_bass_kernel_spmd
from concourse.tile import TileContext
from concourse.vector_clock import ScopedClock

f32 = mybir.dt.float32
AF = mybir.ActivationFunctionType
OP = mybir.AluOpType
AX = mybir.AxisListType

T_FULL = 99800
N = 69
NCORES = 8
EPS = 1e-5
SCAN_B = 499          # 499 * 200 == 99800 exactly
MROWS_A = 1600        # W_in rows per core
MROWS_C = 596         # W_d2 rows per core (8*596=4768 >= 4761)


# ---------------------------------------------------------------------------
# walrus workaround: CTRL-type instructions accept at most 1 sem wait, but the
# TileContext tail drain gets one wait per active proc. Split across drains.
def _patched_drain_and_barrier(self, tick_clock, wait_clock):
    drain_inst = self.nc.sync.drain()
    wait_clock.add_sem_waits(
        drain_inst.ins, ScopedClock({None: tick_clock.global_clock})
    )
    si = drain_inst.ins.sync_info
    waits = list(si.on_wait) if si is not None else []
    if len(waits) > 1:
        drain_inst.ins.sync_info = bass_rust.SyncInfo(
            on_wait=waits[:1], on_update=list(si.on_update)
        )
        for i in range(1, len(waits)):
            extra = self.nc.sync.drain()
            extra.ins.sync_info = bass_rust.SyncInfo(
                on_wait=waits[i : i + 1], on_update=[]
            )
    self.nc.all_engine_barrier()
    assert self.sems is not None
    popped = self.nc._tile_sem_poison_stack.pop()
    assert popped is self._sem_poison
    self.nc.clear_and_free_semaphores(list(self.sems.allocated().values()))
    self.nc.all_engine_barrier()


TileContext._drain_and_barrier = _patched_drain_and_barrier
# ---------------------------------------------------------------------------


def _split_excess_waits(nc, max_waits=1):
    """This walrus build accepts at most one sem wait per instruction; move
    excess waits onto single-wait NOPs inserted just before the owner."""
    n_split = 0
    for f in nc.m.functions:
        for b in f.blocks:
            insts = list(b.instructions)
            out = []
            changed = False
            for inst in insts:
                si = inst.sync_info
                waits = list(si.on_wait) if si is not None else []
                if len(waits) > max_waits:
                    changed = True
                    for i, w in enumerate(waits[max_waits:]):
                        nop = mybir.InstNoOp(
                            name=f"wsp_{inst.name}_{i}", ins=[], outs=[])
                        nop.engine = inst.engine
                        nop.sync_info = bass_rust.SyncInfo(
                            on_wait=[w], on_update=[])
                        out.append(nop)
                        n_split += 1
                    inst.sync_info = bass_rust.SyncInfo(
                        on_wait=waits[:max_waits], on_update=list(si.on_update))
                out.append(inst)
            if changed:
                b.instructions = out
    return n_split


def _reorder_waits(nc):
    """Keep a cross-engine wait on each instruction (engine-level wait, does
    not hold the SEQ); same-engine waits (trivially satisfied in-order but
    kept for HW pipeline-hazard protection) go to the NoOp splits."""
    import collections

    updaters = collections.defaultdict(set)
    for fn in nc.m.functions:
        for b in fn.blocks:
            for i in b.instructions:
                si = i.sync_info
                if si is None:
                    continue
                for u in si.on_update:
                    updaters[u.id].add(i.engine)
    for fn in nc.m.functions:
        for b in fn.blocks:
            for i in b.instructions:
                si = i.sync_info
                if si is None or len(si.on_wait) <= 1:
                    continue
                waits = list(si.on_wait)
                cross = [w for w in waits
                         if updaters.get(w.id, set()) - {i.engine}]
                same = [w for w in waits
                        if not (updaters.get(w.id, set()) - {i.engine})]
                if cross:
                    new = cross + same
                    if [w.id for w in new] != [w.id for w in waits]:
                        i.sync_info = bass_rust.SyncInfo(
                            on_wait=new, on_update=list(si.on_update))


def _pad_w5(w5):
    """(1,64,4,4) -> (4,4,64,32) with real weights in out-column 0."""
    t = np.zeros((4, 4, 64, 32), np.float32)
    t[:, :, :, 0:1] = w5.transpose(2, 3, 1, 0)
    return np.ascontiguousarray(t)


def _col_major_pad(v, ncols):
    """(n,) -> (128, ncols) with element m at [m % 128, m // 128], zero pad."""
    out = np.zeros(128 * ncols, np.float32)
    out[: v.shape[0]] = v
    return np.ascontiguousarray(out.reshape(ncols, 128).T)


def build_program(T=T_FULL, with_front=True, with_scan=True):
    nc = bass.Bass()
    nblk = (T + SCAN_B - 1) // SCAN_B
    assert nblk * SCAN_B == T, "T must be a multiple of SCAN_B"

    # ---- inputs ----
    if with_front:
        x_cols = nc.declare_dram_parameter("x_cols", [128, 16], f32, isOutput=False)
        win_t = nc.declare_dram_parameter("win_t", [2048, MROWS_A], f32, isOutput=False)
        bin_c = nc.declare_dram_parameter("bin_c", [128, 13], f32, isOutput=False)
        w1t = nc.declare_dram_parameter("w1t", [4, 4, 512, 512], f32, isOutput=False)
        w2t = nc.declare_dram_parameter("w2t", [4, 4, 512, 256], f32, isOutput=False)
        w3t = nc.declare_dram_parameter("w3t", [4, 4, 256, 128], f32, isOutput=False)
        w4t = nc.declare_dram_parameter("w4t", [4, 4, 128, 64], f32, isOutput=False)
        w5t = nc.declare_dram_parameter("w5t", [4, 4, 64, 32], f32, isOutput=False)
        g_all = nc.declare_dram_parameter("g_all", [128, 8], f32, isOutput=False)
        be_all = nc.declare_dram_parameter("be_all", [128, 8], f32, isOutput=False)
        wd2_t = nc.declare_dram_parameter("wd2_t", [6400, MROWS_C], f32, isOutput=False)
        bd2_c = nc.declare_dram_parameter("bd2_c", [128, 5], f32, isOutput=False)
    s0_in = nc.declare_dram_parameter("s0", [N, 1], f32, isOutput=False)
    ident_in = nc.declare_dram_parameter("ident", [128, 128], f32, isOutput=False)
    if with_scan:
        out_traj = nc.declare_dram_parameter("out", [T, N], f32, isOutput=True)
    else:
        w_out = nc.declare_dram_parameter("w_out", [N, N], f32, isOutput=True)

    # ---- internal DRAM ----
    if with_front:
        h_shard = nc.dram_tensor("h_shard", [MROWS_A], f32)
        h_full = nc.dram_tensor(
            "h_full", [NCORES * MROWS_A], f32, addr_space="Shared")
        c_scr = nc.dram_tensor("c_scr", [32, 6400], f32)
        wd_shard = nc.dram_tensor("wd_shard", [MROWS_C], f32)
        w_full = nc.dram_tensor(
            "w_full", [NCORES * MROWS_C], f32, addr_space="Shared")
    else:
        w_full = nc.declare_dram_parameter(
            "w_full_in", [NCORES * MROWS_C], f32, isOutput=False)

    with TileContext(nc) as tc:
      if with_front:
        # ================= Phase A: h = W_in @ x + b_in (sharded) ==========
        with (
            tc.tile_pool(name="a_const", bufs=1) as acp,
            tc.tile_pool(name="a_slab", bufs=2) as asp,
            tc.tile_pool(name="a_ps", bufs=1, space="PSUM") as aps,
        ):
            xc = acp.tile([128, 16], f32)
            nc.sync.dma_start(out=xc[:, :], in_=x_cols[:, :])
            bc = acp.tile([128, 13], f32)
            nc.sync.dma_start(out=bc[:, :], in_=bin_c[:, :])
            hc = acp.tile([128, 13], f32)
            for jlo, jhi in ((0, 8), (8, 13)):
                ptiles = {}
                for j in range(jlo, jhi):
                    pt = aps.tile([128, 1], f32, tag=f"hps{j - jlo}", name=f"hps{j}")
                    ptiles[j] = pt
                for k in range(16):
                    gw = min(128 * jhi, MROWS_A) - 128 * jlo
                    slab = asp.tile([128, 1024], f32, tag="aslab")
                    nc.sync.dma_start(
                        out=slab[:, :gw],
                        in_=win_t[128 * k : 128 * (k + 1),
                                  128 * jlo : 128 * jlo + gw])
                    for j in range(jlo, jhi):
                        cj = 128 if j < 12 else 64
                        jj = j - jlo
                        nc.tensor.matmul(
                            ptiles[j][:cj, :],
                            slab[:, 128 * jj : 128 * jj + cj],
                            xc[:, k : k + 1],
                            start=(k == 0),
                            stop=(k == 15),
                        )
                for j in range(jlo, jhi):
                    cj = 128 if j < 12 else 64
                    nc.vector.tensor_tensor(
                        out=hc[:cj, j : j + 1], in0=ptiles[j][:cj, :],
                        in1=bc[:cj, j : j + 1], op=OP.add)
            for j in range(13):
                cj = 128 if j < 12 else 64
                nc.sync.dma_start(
                    out=h_shard[128 * j : 128 * j + cj], in_=hc[:cj, j])
        nc.gpsimd.collective_compute(
            "AllGather", OP.bypass, replica_groups=[list(range(NCORES))],
            ins=[h_shard[:]], outs=[h_full[:]])

        # ================= Phase B: conv stack (replicated) ================
        _lvl = 9  # all conv layers (bisection gates left in place, fully on)
        h2d = h_full.rearrange("(c hw) -> c hw", hw=25)
        gsl = {1: (0, 4), 2: (4, 2), 3: (6, 1), 4: (7, 1)}  # (col offset, ncols)

        with (
            tc.tile_pool(name="bn_const", bufs=1) as bnp,
            tc.tile_pool(name="conv_ps", bufs=1, space="PSUM") as bps,
        ):
            g_sb = bnp.tile([128, 8], f32)
            nc.sync.dma_start(out=g_sb[:, :], in_=g_all[:, :])
            be_sb = bnp.tile([128, 8], f32)
            nc.sync.dma_start(out=be_sb[:, :], in_=be_all[:, :])

            def bn_relu(raw, hw, cch, lidx, j, out_ap):
                """BatchNorm(train) + ReLU from raw (cch,hw) into out_ap."""
                with tc.tile_pool(name=f"bn{lidx}_{j}", bufs=1) as p:
                    s1 = p.tile([cch, 1], f32, tag="s1")
                    nc.vector.tensor_reduce(s1[:, :], raw, axis=AX.X, op=OP.add)
                    mean = p.tile([cch, 1], f32, tag="mean")
                    nc.vector.tensor_scalar_mul(mean[:, :], s1[:, :], 1.0 / hw)
                    sq = p.tile([cch, hw], f32, tag="sq")
                    nc.vector.tensor_tensor(out=sq[:, :], in0=raw, in1=raw, op=OP.mult)
                    s2 = p.tile([cch, 1], f32, tag="s2")
                    nc.vector.tensor_reduce(s2[:, :], sq[:, :], axis=AX.X, op=OP.add)
                    ex2 = p.tile([cch, 1], f32, tag="ex2")
                    nc.vector.tensor_scalar_mul(ex2[:, :], s2[:, :], 1.0 / hw)
                    msq = p.tile([cch, 1], f32, tag="msq")
                    nc.vector.tensor_tensor(
                        out=msq[:, :], in0=mean[:, :], in1=mean[:, :], op=OP.mult)
                    var = p.tile([cch, 1], f32, tag="var")
                    nc.vector.tensor_tensor(
                        out=var[:, :], in0=ex2[:, :], in1=msq[:, :], op=OP.subtract)
                    vps = p.tile([cch, 1], f32, tag="vps")
                    nc.vector.tensor_scalar_add(vps[:, :], var[:, :], EPS)
                    sd = p.tile([cch, 1], f32, tag="sd")
                    nc.scalar.activation(sd[:, :], vps[:, :], AF.Sqrt)
                    rstd = p.tile([cch, 1], f32, tag="rstd")
                    nc.vector.reciprocal(rstd[:, :], sd[:, :])
                    co, _ = gsl[lidx]
                    scale = p.tile([cch, 1], f32, tag="scale")
                    nc.vector.tensor_tensor(
                        out=scale[:, :], in0=g_sb[:cch, co + j : co + j + 1],
                        in1=rstd[:, :], op=OP.mult)
                    t1 = p.tile([cch, 1], f32, tag="t1")
                    nc.vector.tensor_tensor(
                        out=t1[:, :], in0=mean[:, :], in1=scale[:, :], op=OP.mult)
                    bia = p.tile([cch, 1], f32, tag="bia")
                    nc.vector.tensor_tensor(
                        out=bia[:, :], in0=be_sb[:cch, co + j : co + j + 1],
                        in1=t1[:, :], op=OP.subtract)
                    nc.scalar.activation(
                        out_ap, raw, AF.Relu, bias=bia[:, :], scale=scale[:, :])

            # ---- L1: up2(h:512x5x5)->512x10x10 conv 512->512 ----
            with (
                tc.tile_pool(name="l1_in", bufs=1) as l1i,
                tc.tile_pool(name="l1_w", bufs=2) as l1w,
                tc.tile_pool(name="l1_out", bufs=1) as l1o,
            ):
                pads1 = []
                for j in range(4):
                    hm = l1i.tile([128, 25], f32, tag=f"hm{j}")
                    nc.sync.dma_start(out=hm[:, :], in_=h2d[128 * j : 128 * (j + 1), :])
                    pad = l1i.tile([128, 13 * 13], f32, tag=f"pad1_{j}")
                    nc.vector.memset(pad[:, :], 0.0)
                    pv = pad[:, :].rearrange("c (h w) -> c h w", h=13)
                    hv = hm[:, :].rearrange("c (h w) -> c h w", h=5)
                    for a in range(2):
                        for b in range(2):
                            nc.vector.tensor_copy(
                                pv[:, a + 1 : a + 11 : 2, b + 1 : b + 11 : 2], hv[:, :, :])
                    pads1.append(pad)
                ps1s = []
                for jo in range(4):
                    p1 = bps.tile([128, 100], f32, tag=f"l1ps{jo}", name=f"l1ps{jo}")
                    ps1s.append(p1)
                nmm = 0
                for ji in range(4):
                    for dy in range(4):
                        for dx in range(4):
                            slab = l1w.tile([128, 512], f32, tag="w1slab")
                            nc.sync.dma_start(
                                out=slab[:, :],
                                in_=w1t[dy, dx, 128 * ji : 128 * (ji + 1), :])
                            rhs = pads1[ji][:, :].rearrange(
                                "c (h w) -> c h w", h=13)[:, dy : dy + 10, dx : dx + 10]
                            for jo in range(4):
                                nc.tensor.matmul(
                                    ps1s[jo][:, :],
                                    slab[:, 128 * jo : 128 * (jo + 1)], rhs,
                                    start=(nmm == 0), stop=(nmm == 63))
                            nmm += 1
                pads2 = []
                for jo in range(4):
                    raw = l1o.tile([128, 100], f32, tag=f"raw1_{jo}")
                    nc.vector.tensor_copy(raw[:, :], ps1s[jo][:, :])
                    relu = l1o.tile([128, 100], f32, tag=f"relu1_{jo}")
                    bn_relu(raw[:, :], 100, 128, 1, jo, relu[:, :])
                    pad = l1o.tile([128, 23 * 23], f32, tag=f"pad2_{jo}")
                    nc.vector.memset(pad[:, :], 0.0)
                    pv = pad[:, :].rearrange("c (h w) -> c h w", h=23)
                    rv = relu[:, :].rearrange("c (h w) -> c h w", h=10)
                    for a in range(2):
                        for b in range(2):
                            nc.vector.tensor_copy(
                                pv[:, a + 1 : a + 21 : 2, b + 1 : b + 21 : 2], rv[:, :, :])
                    pads2.append(pad)

                if _lvl >= 2:
                  # ---- L2: 512x20x20 conv 512->256 ----
                  with (
                      tc.tile_pool(name="l2_w", bufs=2) as l2w,
                      tc.tile_pool(name="l2_out", bufs=1) as l2o,
                  ):
                      psA = bps.tile([128, 400], f32, tag="cpsA")
                      psB = bps.tile([128, 400], f32, tag="cpsB")
                      nmm = 0
                      for ji in range(4):
                          for dy in range(4):
                              for dx in range(4):
                                  slab = l2w.tile([128, 256], f32, tag="w2slab")
                                  nc.sync.dma_start(
                                      out=slab[:, :],
                                      in_=w2t[dy, dx, 128 * ji : 128 * (ji + 1), :])
                                  rhs = pads2[ji][:, :].rearrange(
                                      "c (h w) -> c h w", h=23)[:, dy : dy + 20, dx : dx + 20]
                                  nc.tensor.matmul(
                                      psA[:, :], slab[:, 0:128], rhs,
                                      start=(nmm == 0), stop=(nmm == 63))
                                  nc.tensor.matmul(
                                      psB[:, :], slab[:, 128:256], rhs,
                                      start=(nmm == 0), stop=(nmm == 63))
                                  nmm += 1
                      pads3 = []
                      for jo, ps in enumerate((psA, psB)):
                          raw = l2o.tile([128, 400], f32, tag=f"raw2_{jo}")
                          nc.vector.tensor_copy(raw[:, :], ps[:, :])
                          relu = l2o.tile([128, 400], f32, tag=f"relu2_{jo}")
                          bn_relu(raw[:, :], 400, 128, 2, jo, relu[:, :])
                          pad = l2o.tile([128, 43 * 43], f32, tag=f"pad3_{jo}")
                          nc.vector.memset(pad[:, :], 0.0)
                          pv = pad[:, :].rearrange("c (h w) -> c h w", h=43)
                          rv = relu[:, :].rearrange("c (h w) -> c h w", h=20)
                          for a in range(2):
                              for b in range(2):
                                  nc.vector.tensor_copy(
                                      pv[:, a + 1 : a + 41 : 2, b + 1 : b + 41 : 2],
                                      rv[:, :, :])
                          pads3.append(pad)

                      if _lvl >= 3:
                        # ---- L3: 256x40x40 conv 256->128 ----
                        with (
                            tc.tile_pool(name="l3_w", bufs=1) as l3w,
                            tc.tile_pool(name="l3_out", bufs=1) as l3o,
                        ):
                            wsl3 = l3w.tile([128, 32 * 128], f32)
                            for ji in range(2):
                                for dy in range(4):
                                    for dx in range(4):
                                        si = (ji * 16 + dy * 4 + dx) * 128
                                        nc.sync.dma_start(
                                            out=wsl3[:, si : si + 128],
                                            in_=w3t[dy, dx, 128 * ji : 128 * (ji + 1), :])
                            raw3 = l3o.tile([128, 1600], f32)
                            for st in range(4):
                                ps = bps.tile([128, 400], f32, tag="cps", bufs=2)
                                nmm = 0
                                for ji in range(2):
                                    for dy in range(4):
                                        for dx in range(4):
                                            si = (ji * 16 + dy * 4 + dx) * 128
                                            rhs = pads3[ji][:, :].rearrange(
                                                "c (h w) -> c h w", h=43)[
                                                :, st * 10 + dy : st * 10 + dy + 10,
                                                dx : dx + 40]
                                            nc.tensor.matmul(
                                                ps[:, :], wsl3[:, si : si + 128], rhs,
                                                start=(nmm == 0), stop=(nmm == 31))
                                            nmm += 1
                                nc.vector.tensor_copy(
                                    raw3[:, 400 * st : 400 * (st + 1)], ps[:, :])
                            relu3 = l3o.tile([128, 1600], f32)
                            bn_relu(raw3[:, :], 1600, 128, 3, 0, relu3[:, :])
                            pad4 = l3o.tile([128, 83 * 83], f32)
                            nc.vector.memset(pad4[:, :], 0.0)
                            pv = pad4[:, :].rearrange("c (h w) -> c h w", h=83)
                            rv = relu3[:, :].rearrange("c (h w) -> c h w", h=40)
                            for a in range(2):
                                for b in range(2):
                                    nc.vector.tensor_copy(
                                        pv[:, a + 1 : a + 81 : 2, b + 1 : b + 81 : 2],
                                        rv[:, :, :])

                            if _lvl >= 4:
                              # ---- L4: 128x80x80 conv 128->64 ----
                              with (
                                  tc.tile_pool(name="l4_w", bufs=1) as l4w,
                                  tc.tile_pool(name="l4_out", bufs=1) as l4o,
                              ):
                                  wsl4 = l4w.tile([128, 16 * 64], f32)
                                  for dy in range(4):
                                      for dx in range(4):
                                          si = (dy * 4 + dx) * 64
                                          nc.sync.dma_start(
                                              out=wsl4[:, si : si + 64],
                                              in_=w4t[dy, dx, :, :])
                                  raw4 = l4o.tile([64, 6400], f32)
                                  for st in range(16):
                                      ps = bps.tile([64, 400], f32, tag="cps", bufs=2)
                                      nmm = 0
                                      for dy in range(4):
                                          for dx in range(4):
                                              si = (dy * 4 + dx) * 64
                                              rhs = pad4[:, :].rearrange(
                                                  "c (h w) -> c h w", h=83)[
                                                  :, st * 5 + dy : st * 5 + dy + 5,
                                                  dx : dx + 80]
                                              nc.tensor.matmul(
                                                  ps[:, :], wsl4[:, si : si + 64], rhs,
                                                  start=(nmm == 0), stop=(nmm == 15))
                                              nmm += 1
                                      nc.vector.tensor_copy(
                                          raw4[:, 400 * st : 400 * (st + 1)], ps[:, :])
                                  pad5 = l4o.tile([64, 83 * 83], f32)
                                  nc.vector.memset(pad5[:, :], 0.0)
                                  pv5 = pad5[:, :].rearrange("c (h w) -> c h w", h=83)[
                                      :, 1:81, 1:81]
                                  bn_relu(raw4[:, :], 6400, 64, 4, 0, pv5)

                                  if _lvl >= 5:
                                    # ---- L5: 64x80x80 conv 64->1 + tanh -> c ----
                                    with (
                                        tc.tile_pool(name="l5_w", bufs=1) as l5w,
                                        tc.tile_pool(name="l5_out", bufs=1) as l5o,
                                    ):
                                        wsl5 = l5w.tile([64, 16 * 32], f32)
                                        for dy in range(4):
                                            for dx in range(4):
                                                _p5 = (dy * 4 + dx) * 32
                                                nc.sync.dma_start(
                                                    out=wsl5[:, _p5 : _p5 + 32],
                                                    in_=w5t[dy, dx, :, :])
                                        for st in range(16):
                                            ps = bps.tile([32, 400], f32, tag="cps", bufs=2)
                                            nmm = 0
                                            for dy in range(4):
                                                for dx in range(4):
                                                    rhs = pad5[:, :].rearrange(
                                                        "c (h w) -> c h w", h=83)[
                                                        :, st * 5 + dy : st * 5 + dy + 5,
                                                        dx : dx + 80]
                                                    _p5 = (dy * 4 + dx) * 32
                                                    nc.tensor.matmul(
                                                        ps[:, :],
                                                        wsl5[:, _p5 : _p5 + 32],
                                                        rhs,
                                                        start=(nmm == 0), stop=(nmm == 15))
                                                    nmm += 1
                                            c32 = l5o.tile([32, 400], f32, tag="c32", name=f"c32_{st}")
                                            nc.scalar.activation(c32[:, :], ps[:, :], AF.Tanh)
                                            nc.sync.dma_start(
                                                out=c_scr[:, 400 * st : 400 * (st + 1)], in_=c32[:, :])

        # ================= Phase C: w = W_d2 @ c + b_d2 (sharded) ==========
        _skip_c = False
        if not _skip_c:
          with (
              tc.tile_pool(name="c_const", bufs=1) as ccp,
              tc.tile_pool(name="c_slab", bufs=2) as csp,
              tc.tile_pool(name="c_ps", bufs=1, space="PSUM") as cps,
          ):
              c_cols = ccp.tile([128, 50], f32)
              nc.sync.dma_start(
                  out=c_cols[:, :], in_=c_scr[0, :].rearrange("(f p) -> p f", p=128))
              bdc = ccp.tile([128, 5], f32)
              nc.sync.dma_start(out=bdc[:, :], in_=bd2_c[:, :])
              wtiles = {}
              for j in range(5):
                  wt_ps = cps.tile([128, 1], f32, tag=f"wps{j}", name=f"wps{j}")
                  wtiles[j] = wt_ps
              for k in range(50):
                  slab = csp.tile([128, MROWS_C], f32, tag="cslab")
                  nc.sync.dma_start(
                      out=slab[:, :], in_=wd2_t[128 * k : 128 * (k + 1), :])
                  for j in range(5):
                      cj = 128 if j < 4 else 84
                      nc.tensor.matmul(
                          wtiles[j][:cj, :], slab[:, 128 * j : 128 * j + cj],
                          c_cols[:, k : k + 1], start=(k == 0), stop=(k == 49))
              wdc = ccp.tile([128, 5], f32)
              for j in range(5):
                  cj = 128 if j < 4 else 84
                  nc.vector.tensor_tensor(
                      out=wdc[:cj, j : j + 1], in0=wtiles[j][:cj, :],
                      in1=bdc[:cj, j : j + 1], op=OP.add)
              for j in range(5):
                  cj = 128 if j < 4 else 84
                  nc.sync.dma_start(
                      out=wd_shard[128 * j : 128 * j + cj], in_=wdc[:cj, j])
        if not _skip_c:
            nc.gpsimd.collective_compute(
                "AllGather", OP.bypass, replica_groups=[list(range(NCORES))],
                ins=[wd_shard[:]], outs=[w_full[:]])

      if not with_scan:
          with tc.tile_pool(name="wout", bufs=1) as wop:
              w_sb0 = wop.tile([N, N], f32)
              nc.sync.dma_start(
                  out=w_sb0[:, :],
                  in_=w_full[0 : N * N].rearrange("(j i) -> j i", i=N))
              nc.sync.dma_start(out=w_out[:, :], in_=w_sb0[:, :])

      # ================= Phase D: spiking scan =========================
      # Column k of sblk holds s_{t0+k+1} (= output row t0+k); column k of
      # ublk holds u_{t0+k+1}.  Steady-state iteration k (t = t0 + k):
      #   DVE : sblk[k] = u_t - s_t            (output, off critical path)
      #   PE  : y_ps  = (-w).s_t  [start]      (s_t is a full cycle old)
      #         y_ps += ( w).u_t  [stop]       (u_t is the critical input)
      #   Act : ublk[k] = tanh(y_ps) = u_{t+1}
      if with_scan:
        # transpose chunking of one SCAN_B block (chunks of <=128 columns)
        chunks = []
        c0 = 0
        while c0 < SCAN_B:
            cw = min(128, SCAN_B - c0)
            chunks.append((c0, cw))
            c0 += cw
        with (
            tc.tile_pool(name="d_const", bufs=1) as dcp,
            tc.tile_pool(name="d_state", bufs=1) as dsp,
            tc.tile_pool(name="d_blk", bufs=3) as dbp,
            tc.tile_pool(name="d_out", bufs=2 * len(chunks)) as dop,
            tc.tile_pool(name="d_ps", bufs=4, space="PSUM") as dps,
            tc.tile_pool(name="t_ps", bufs=2, space="PSUM") as tpp,
        ):
            w_sb = dcp.tile([N, N], f32)
            nc.sync.dma_start(
                out=w_sb[:, :],
                in_=w_full[0 : N * N].rearrange("(j i) -> j i", i=N))
            wneg_sb = dcp.tile([N, N], f32)
            nc.vector.tensor_scalar_mul(wneg_sb[:, :], w_sb[:, :], -1.0)
            ident = dcp.tile([128, 128], f32)
            nc.sync.dma_start(out=ident[:, :], in_=ident_in[:, :])
            s_col = dsp.tile([N, 1], f32)
            nc.sync.dma_start(out=s_col[:, :], in_=s0_in[:, :])
            u_col = dsp.tile([N, 1], f32)

            # prime: u_0 = tanh(w . s_0)
            y0 = dps.tile([N, 1], f32, tag="y")
            nc.tensor.matmul(
                y0[:, :], w_sb[:, :], s_col[:, :], start=True, stop=True)
            nc.scalar.activation(u_col[:, :], y0[:, :], AF.Tanh)

            with tc.For_i(
                0, T, SCAN_B,
                hint_engines=(
                    mybir.EngineType.PE, mybir.EngineType.Activation,
                    mybir.EngineType.DVE),
            ) as iv:
                sblk = dbp.tile([N, SCAN_B], f32, tag="sblk")
                ublk = dbp.tile([N, SCAN_B], f32, tag="ublk")
                for k in range(SCAN_B):
                    s_prev = s_col[:, :] if k == 0 else sblk[:, k - 1 : k]
                    u_prev = u_col[:, :] if k == 0 else ublk[:, k - 1 : k]
                    # output row t: s_{t+1} = u_t - s_t  (DVE)
                    nc.vector.tensor_tensor(
                        out=sblk[:, k : k + 1], in0=u_prev, in1=s_prev,
                        op=OP.subtract)
                    # y_{t+1} = w.u_t - w.s_t  (PE; mm_s first, it is old)
                    y_ps = dps.tile([N, 1], f32, tag="y")
                    nc.tensor.matmul(
                        y_ps[:, :], wneg_sb[:, :], s_prev,
                        start=True, stop=False)
                    nc.tensor.matmul(
                        y_ps[:, :], w_sb[:, :], u_prev,
                        start=False, stop=True)
                    # u_{t+1} = tanh(y_{t+1})  (Act)
                    nc.scalar.activation(
                        ublk[:, k : k + 1], y_ps[:, :], AF.Tanh)
                # carry state into the next block (same-engine, in-order)
                nc.scalar.copy(u_col[:, :], ublk[:, SCAN_B - 1 : SCAN_B])
                nc.vector.tensor_copy(
                    s_col[:, :], sblk[:, SCAN_B - 1 : SCAN_B])
                # transpose this block's outputs to (T, 69) rows and DMA out
                for c0, cw in chunks:
                    tps = tpp.tile([128, N], f32, tag="tps")
                    nc.tensor.transpose(
                        tps[:cw, :], sblk[:, c0 : c0 + cw], ident[:N, :N])
                    tsb = dop.tile([128, N], f32, tag=f"tsb{c0}")
                    nc.vector.tensor_copy(tsb[:cw, :], tps[:cw, :])
                    nc.sync.dma_start(
                        out=out_traj[bass.ds(iv + c0, cw), :],
                        in_=tsb[:cw, :])

    return nc


def _marshal_inputs(inputs):
    """Build the 8 per-core input maps from the full problem inputs."""
    x = np.asarray(inputs["x"], np.float32).reshape(2048)
    win = np.asarray(inputs["W_in"], np.float32)
    b_in = np.asarray(inputs["b_in"], np.float32)
    wd2 = np.asarray(inputs["W_d2"], np.float32)
    bd2 = np.asarray(inputs["b_d2"], np.float32)
    sp = np.asarray(inputs["start_part"], np.float32)

    x_cols = np.ascontiguousarray(x.reshape(16, 128).T)
    g_all = np.zeros((128, 8), np.float32)
    be_all = np.zeros((128, 8), np.float32)
    g_all[:, 0:4] = _col_major_pad(np.asarray(inputs["g1"], np.float32), 4)
    g_all[:, 4:6] = _col_major_pad(np.asarray(inputs["g2"], np.float32), 2)
    g_all[:, 6:7] = _col_major_pad(np.asarray(inputs["g3"], np.float32), 1)
    g_all[:, 7:8] = _col_major_pad(np.asarray(inputs["g4"], np.float32), 1)
    be_all[:, 0:4] = _col_major_pad(np.asarray(inputs["be1"], np.float32), 4)
    be_all[:, 4:6] = _col_major_pad(np.asarray(inputs["be2"], np.float32), 2)
    be_all[:, 6:7] = _col_major_pad(np.asarray(inputs["be3"], np.float32), 1)
    be_all[:, 7:8] = _col_major_pad(np.asarray(inputs["be4"], np.float32), 1)
    wts = {
        "w1t": np.ascontiguousarray(
            np.asarray(inputs["w1"], np.float32).transpose(2, 3, 1, 0)),
        "w2t": np.ascontiguousarray(
            np.asarray(inputs["w2"], np.float32).transpose(2, 3, 1, 0)),
        "w3t": np.ascontiguousarray(
            np.asarray(inputs["w3"], np.float32).transpose(2, 3, 1, 0)),
        "w4t": np.ascontiguousarray(
            np.asarray(inputs["w4"], np.float32).transpose(2, 3, 1, 0)),
        "w5t": _pad_w5(np.asarray(inputs["w5"], np.float32)),
    }
    s0 = np.ascontiguousarray(sp[-1].reshape(N, 1))
    ident = np.eye(128, dtype=np.float32)

    wd2_pad = np.zeros((NCORES * MROWS_C, 6400), np.float32)
    wd2_pad[: wd2.shape[0]] = wd2
    bd2_pad = np.zeros(NCORES * MROWS_C, np.float32)
    bd2_pad[: bd2.shape[0]] = bd2

    in_maps = []
    for c in range(NCORES):
        m = {
            "x_cols": x_cols,
            "win_t": np.ascontiguousarray(
                win[MROWS_A * c : MROWS_A * (c + 1)].T),
            "bin_c": _col_major_pad(b_in[MROWS_A * c : MROWS_A * (c + 1)], 13),
            "g_all": g_all,
            "be_all": be_all,
            "wd2_t": np.ascontiguousarray(
                wd2_pad[MROWS_C * c : MROWS_C * (c + 1)].T),
            "bd2_c": _col_major_pad(bd2_pad[MROWS_C * c : MROWS_C * (c + 1)], 5),
            "s0": s0,
            "ident": ident,
        }
        m.update(wts)
        in_maps.append(m)
    return in_maps


LAST_EXEC_NS = None


def kernel(**inputs) -> np.ndarray:
    global LAST_EXEC_NS
    import os

    trace = bool(os.environ.get("KERNEL_TRACE"))
    nc = build_program(T_FULL)
    _reorder_waits(nc)
    _split_excess_waits(nc)
    in_maps = _marshal_inputs(inputs)
    res = run_bass_kernel_spmd(nc, in_maps, list(range(NCORES)), trace=trace)
    if res.exec_time_ns is not None:
        LAST_EXEC_NS = res.exec_time_ns
    out = np.asarray(res.results[0]["out"], np.float32)
    return out.reshape(1, T_FULL, N)


if __name__ == "__main__":
    # CoreSim selftest with a short scan (no hardware needed).
    import sys
    import time

    T_test = SCAN_B * 2
    nc = build_program(T_test)
    print("program built", flush=True)

    sys.path.insert(0, "/root/problem")
    import jax
    jax.config.update("jax_platform_name", "cpu")
    import reference

    inputs = reference.setup_inputs()
    inputs = {k: np.asarray(v) for k, v in inputs.items()}
    in_maps = _marshal_inputs(inputs)

    from concourse.bass_interp import MultiCoreSim

    t0 = time.time()
    sim = MultiCoreSim(nc, NCORES)
    for i in range(NCORES):
        for k, v in in_maps[i].items():
            sim.cores[i].tensor(k)[:] = v
    sim.simulate()
    print("sim time", time.time() - t0, flush=True)
    got = np.array(sim.cores[0].tensor("out"))

    # host reference for the short horizon
    w = np.load("/tmp/w.npy")
    s = np.asarray(inputs["start_part"])[-1].astype(np.float32)
    ref = np.empty((T_test, N), np.float32)
    for t in range(T_test):
        s = (np.tanh((s @ w).astype(np.float32)).astype(np.float32) - s).astype(
            np.float32)
        ref[t] = s
    err = np.abs(got - ref)
    rel = np.abs(got - ref) / (np.abs(ref) + 1e-6)
    print("traj absmax err:", err.max(), "rel max:", rel.max())
    print("first rows got:", got[0, :4], "ref:", ref[0, :4])



# revision 13
# speedup vs baseline: 1.1984x; 1.0178x over previous
"""Trainium2 Bass kernel for nn_DCGAN_G (DCGAN generator + 69-neuron spiking scan).

Strategy (8 NeuronCores, SPMD):
  A. W_in matvec (12800x2048) row-sharded 8x -> AllGather h1 (12800).
  B. DCGAN conv stack replicated on every core (tiny: ~3 GMAC).
  C. W_d2 matvec (4761x6400) row-sharded 8x -> AllGather w (69x69).
  D. 99800-step spiking recurrence (inherently serial).  Critical cycle is
     tanh -> matmul -> tanh only: with u_t = tanh(y_t) the next matvec is
     computed as y_{t+1} = w.u_t - w.s_t (two accumulating matmuls; the
     second operand s_t is a full step old), so the elementwise subtract
     s_{t+1} = u_t - s_t runs on DVE OFF the critical path.  Sem-wait
     hygiene keeps the one critical wait on each instruction (engine-level
     wait) and NoOp-splits only early-satisfied waits.  Output rows are
     PE-transposed in 128-column chunks inside the loop (PE is otherwise
     idle), so no separate transpose phase and no DRAM round-trip.
"""
import numpy as np

import bass_rust
import concourse.bass as bass
import concourse.mybir as mybir
from concourse.bass_utils import run_bass_kernel_spmd
from concourse.tile import TileContext
from concourse.vector_clock import ScopedClock

f32 = mybir.dt.float32
AF = mybir.ActivationFunctionType
OP = mybir.AluOpType
AX = mybir.AxisListType

T_FULL = 99800
N = 69
NCORES = 8
EPS = 1e-5
SCAN_B = 1996         # 1996 * 50 == 99800 exactly
MROWS_A = 1600        # W_in rows per core
MROWS_C = 596         # W_d2 rows per core (8*596=4768 >= 4761)


# ---------------------------------------------------------------------------
# walrus workaround: CTRL-type instructions accept at most 1 sem wait, but the
# TileContext tail drain gets one wait per active proc. Split across drains.
def _patched_drain_and_barrier(self, tick_clock, wait_clock):
    drain_inst = self.nc.sync.drain()
    wait_clock.add_sem_waits(
        drain_inst.ins, ScopedClock({None: tick_clock.global_clock})
    )
    si = drain_inst.ins.sync_info
    waits = list(si.on_wait) if si is not None else []
    if len(waits) > 1:
        drain_inst.ins.sync_info = bass_rust.SyncInfo(
            on_wait=waits[:1], on_update=list(si.on_update)
        )
        for i in range(1, len(waits)):
            extra = self.nc.sync.drain()
            extra.ins.sync_info = bass_rust.SyncInfo(
                on_wait=waits[i : i + 1], on_update=[]
            )
    self.nc.all_engine_barrier()
    assert self.sems is not None
    popped = self.nc._tile_sem_poison_stack.pop()
    assert popped is self._sem_poison
    self.nc.clear_and_free_semaphores(list(self.sems.allocated().values()))
    self.nc.all_engine_barrier()


TileContext._drain_and_barrier = _patched_drain_and_barrier
# ---------------------------------------------------------------------------


def _split_excess_waits(nc, max_waits=1):
    """This walrus build accepts at most one sem wait per instruction; move
    excess waits onto single-wait NOPs inserted just before the owner."""
    n_split = 0
    for f in nc.m.functions:
        for b in f.blocks:
            insts = list(b.instructions)
            out = []
            changed = False
            for inst in insts:
                si = inst.sync_info
                waits = list(si.on_wait) if si is not None else []
                if len(waits) > max_waits:
                    changed = True
                    for i, w in enumerate(waits[max_waits:]):
                        nop = mybir.InstNoOp(
                            name=f"wsp_{inst.name}_{i}", ins=[], outs=[])
                        nop.engine = inst.engine
                        nop.sync_info = bass_rust.SyncInfo(
                            on_wait=[w], on_update=[])
                        out.append(nop)
                        n_split += 1
                    inst.sync_info = bass_rust.SyncInfo(
                        on_wait=waits[:max_waits], on_update=list(si.on_update))
                out.append(inst)
            if changed:
                b.instructions = out
    return n_split


def _reorder_waits(nc):
    """Keep a cross-engine wait on each instruction (engine-level wait, does
    not hold the SEQ); same-engine waits (trivially satisfied in-order but
    kept for HW pipeline-hazard protection) go to the NoOp splits."""
    import collections

    updaters = collections.defaultdict(set)
    for fn in nc.m.functions:
        for b in fn.blocks:
            for i in b.instructions:
                si = i.sync_info
                if si is None:
                    continue
                for u in si.on_update:
                    updaters[u.id].add(i.engine)
    for fn in nc.m.functions:
        for b in fn.blocks:
            for i in b.instructions:
                si = i.sync_info
                if si is None or len(si.on_wait) <= 1:
                    continue
                waits = list(si.on_wait)
                cross = [w for w in waits
                         if updaters.get(w.id, set()) - {i.engine}]
                same = [w for w in waits
                        if not (updaters.get(w.id, set()) - {i.engine})]
                if cross:
                    new = cross + same
                    if [w.id for w in new] != [w.id for w in waits]:
                        i.sync_info = bass_rust.SyncInfo(
                            on_wait=new, on_update=list(si.on_update))


def _pad_w5(w5):
    """(1,64,4,4) -> (4,4,64,32) with real weights in out-column 0."""
    t = np.zeros((4, 4, 64, 32), np.float32)
    t[:, :, :, 0:1] = w5.transpose(2, 3, 1, 0)
    return np.ascontiguousarray(t)


def _col_major_pad(v, ncols):
    """(n,) -> (128, ncols) with element m at [m % 128, m // 128], zero pad."""
    out = np.zeros(128 * ncols, np.float32)
    out[: v.shape[0]] = v
    return np.ascontiguousarray(out.reshape(ncols, 128).T)


def build_program(T=T_FULL, with_front=True, with_scan=True):
    nc = bass.Bass()
    nblk = (T + SCAN_B - 1) // SCAN_B
    assert nblk * SCAN_B == T, "T must be a multiple of SCAN_B"

    # ---- inputs ----
    if with_front:
        x_cols = nc.declare_dram_parameter("x_cols", [128, 16], f32, isOutput=False)
        win_t = nc.declare_dram_parameter("win_t", [2048, MROWS_A], f32, isOutput=False)
        bin_c = nc.declare_dram_parameter("bin_c", [128, 13], f32, isOutput=False)
        w1t = nc.declare_dram_parameter("w1t", [4, 4, 512, 512], f32, isOutput=False)
        w2t = nc.declare_dram_parameter("w2t", [4, 4, 512, 256], f32, isOutput=False)
        w3t = nc.declare_dram_parameter("w3t", [4, 4, 256, 128], f32, isOutput=False)
        w4t = nc.declare_dram_parameter("w4t", [4, 4, 128, 64], f32, isOutput=False)
        w5t = nc.declare_dram_parameter("w5t", [4, 4, 64, 32], f32, isOutput=False)
        g_all = nc.declare_dram_parameter("g_all", [128, 8], f32, isOutput=False)
        be_all = nc.declare_dram_parameter("be_all", [128, 8], f32, isOutput=False)
        wd2_t = nc.declare_dram_parameter("wd2_t", [6400, MROWS_C], f32, isOutput=False)
        bd2_c = nc.declare_dram_parameter("bd2_c", [128, 5], f32, isOutput=False)
    s0_in = nc.declare_dram_parameter("s0", [N, 1], f32, isOutput=False)
    ident_in = nc.declare_dram_parameter("ident", [128, 128], f32, isOutput=False)
    if with_scan:
        out_traj = nc.declare_dram_parameter("out", [T, N], f32, isOutput=True)
    else:
        w_out = nc.declare_dram_parameter("w_out", [N, N], f32, isOutput=True)

    # ---- internal DRAM ----
    if with_front:
        h_shard = nc.dram_tensor("h_shard", [MROWS_A], f32)
        h_full = nc.dram_tensor(
            "h_full", [NCORES * MROWS_A], f32, addr_space="Shared")
        c_scr = nc.dram_tensor("c_scr", [32, 6400], f32)
        wd_shard = nc.dram_tensor("wd_shard", [MROWS_C], f32)
        w_full = nc.dram_tensor(
            "w_full", [NCORES * MROWS_C], f32, addr_space="Shared")
    else:
        w_full = nc.declare_dram_parameter(
            "w_full_in", [NCORES * MROWS_C], f32, isOutput=False)

    with TileContext(nc) as tc:
      if with_front:
        # ================= Phase A: h = W_in @ x + b_in (sharded) ==========
        with (
            tc.tile_pool(name="a_const", bufs=1) as acp,
            tc.tile_pool(name="a_slab", bufs=2) as asp,
            tc.tile_pool(name="a_ps", bufs=1, space="PSUM") as aps,
        ):
            xc = acp.tile([128, 16], f32)
            nc.sync.dma_start(out=xc[:, :], in_=x_cols[:, :])
            bc = acp.tile([128, 13], f32)
            nc.sync.dma_start(out=bc[:, :], in_=bin_c[:, :])
            hc = acp.tile([128, 13], f32)
            for jlo, jhi in ((0, 8), (8, 13)):
                ptiles = {}
                for j in range(jlo, jhi):
                    pt = aps.tile([128, 1], f32, tag=f"hps{j - jlo}", name=f"hps{j}")
                    ptiles[j] = pt
                for k in range(16):
                    gw = min(128 * jhi, MROWS_A) - 128 * jlo
                    slab = asp.tile([128, 1024], f32, tag="aslab")
                    nc.sync.dma_start(
                        out=slab[:, :gw],
                        in_=win_t[128 * k : 128 * (k + 1),
                                  128 * jlo : 128 * jlo + gw])
                    for j in range(jlo, jhi):
                        cj = 128 if j < 12 else 64
                        jj = j - jlo
                        nc.tensor.matmul(
                            ptiles[j][:cj, :],
                            slab[:, 128 * jj : 128 * jj + cj],
                            xc[:, k : k + 1],
                            start=(k == 0),
                            stop=(k == 15),
                        )
                for j in range(jlo, jhi):
                    cj = 128 if j < 12 else 64
                    nc.vector.tensor_tensor(
                        out=hc[:cj, j : j + 1], in0=ptiles[j][:cj, :],
                        in1=bc[:cj, j : j + 1], op=OP.add)
            for j in range(13):
                cj = 128 if j < 12 else 64
                nc.sync.dma_start(
                    out=h_shard[128 * j : 128 * j + cj], in_=hc[:cj, j])
        nc.gpsimd.collective_compute(
            "AllGather", OP.bypass, replica_groups=[list(range(NCORES))],
            ins=[h_shard[:]], outs=[h_full[:]])

        # ================= Phase B: conv stack (replicated) ================
        _lvl = 9  # all conv layers (bisection gates left in place, fully on)
        h2d = h_full.rearrange("(c hw) -> c hw", hw=25)
        gsl = {1: (0, 4), 2: (4, 2), 3: (6, 1), 4: (7, 1)}  # (col offset, ncols)

        with (
            tc.tile_pool(name="bn_const", bufs=1) as bnp,
            tc.tile_pool(name="conv_ps", bufs=1, space="PSUM") as bps,
        ):
            g_sb = bnp.tile([128, 8], f32)
            nc.sync.dma_start(out=g_sb[:, :], in_=g_all[:, :])
            be_sb = bnp.tile([128, 8], f32)
            nc.sync.dma_start(out=be_sb[:, :], in_=be_all[:, :])

            def bn_relu(raw, hw, cch, lidx, j, out_ap):
                """BatchNorm(train) + ReLU from raw (cch,hw) into out_ap."""
                with tc.tile_pool(name=f"bn{lidx}_{j}", bufs=1) as p:
                    s1 = p.tile([cch, 1], f32, tag="s1")
                    nc.vector.tensor_reduce(s1[:, :], raw, axis=AX.X, op=OP.add)
                    mean = p.tile([cch, 1], f32, tag="mean")
                    nc.vector.tensor_scalar_mul(mean[:, :], s1[:, :], 1.0 / hw)
                    sq = p.tile([cch, hw], f32, tag="sq")
                    nc.vector.tensor_tensor(out=sq[:, :], in0=raw, in1=raw, op=OP.mult)
                    s2 = p.tile([cch, 1], f32, tag="s2")
                    nc.vector.tensor_reduce(s2[:, :], sq[:, :], axis=AX.X, op=OP.add)
                    ex2 = p.tile([cch, 1], f32, tag="ex2")
                    nc.vector.tensor_scalar_mul(ex2[:, :], s2[:, :], 1.0 / hw)
                    msq = p.tile([cch, 1], f32, tag="msq")
                    nc.vector.tensor_tensor(
                        out=msq[:, :], in0=mean[:, :], in1=mean[:, :], op=OP.mult)
                    var = p.tile([cch, 1], f32, tag="var")
                    nc.vector.tensor_tensor(
                        out=var[:, :], in0=ex2[:, :], in1=msq[:, :], op=OP.subtract)
                    vps = p.tile([cch, 1], f32, tag="vps")
                    nc.vector.tensor_scalar_add(vps[:, :], var[:, :], EPS)
                    sd = p.tile([cch, 1], f32, tag="sd")
                    nc.scalar.activation(sd[:, :], vps[:, :], AF.Sqrt)
                    rstd = p.tile([cch, 1], f32, tag="rstd")
                    nc.vector.reciprocal(rstd[:, :], sd[:, :])
                    co, _ = gsl[lidx]
                    scale = p.tile([cch, 1], f32, tag="scale")
                    nc.vector.tensor_tensor(
                        out=scale[:, :], in0=g_sb[:cch, co + j : co + j + 1],
                        in1=rstd[:, :], op=OP.mult)
                    t1 = p.tile([cch, 1], f32, tag="t1")
                    nc.vector.tensor_tensor(
                        out=t1[:, :], in0=mean[:, :], in1=scale[:, :], op=OP.mult)
                    bia = p.tile([cch, 1], f32, tag="bia")
                    nc.vector.tensor_tensor(
                        out=bia[:, :], in0=be_sb[:cch, co + j : co + j + 1],
                        in1=t1[:, :], op=OP.subtract)
                    nc.scalar.activation(
                        out_ap, raw, AF.Relu, bias=bia[:, :], scale=scale[:, :])

            # ---- L1: up2(h:512x5x5)->512x10x10 conv 512->512 ----
            with (
                tc.tile_pool(name="l1_in", bufs=1) as l1i,
                tc.tile_pool(name="l1_w", bufs=2) as l1w,
                tc.tile_pool(name="l1_out", bufs=1) as l1o,
            ):
                pads1 = []
                for j in range(4):
                    hm = l1i.tile([128, 25], f32, tag=f"hm{j}")
                    nc.sync.dma_start(out=hm[:, :], in_=h2d[128 * j : 128 * (j + 1), :])
                    pad = l1i.tile([128, 13 * 13], f32, tag=f"pad1_{j}")
                    nc.vector.memset(pad[:, :], 0.0)
                    pv = pad[:, :].rearrange("c (h w) -> c h w", h=13)
                    hv = hm[:, :].rearrange("c (h w) -> c h w", h=5)
                    for a in range(2):
                        for b in range(2):
                            nc.vector.tensor_copy(
                                pv[:, a + 1 : a + 11 : 2, b + 1 : b + 11 : 2], hv[:, :, :])
                    pads1.append(pad)
                ps1s = []
                for jo in range(4):
                    p1 = bps.tile([128, 100], f32, tag=f"l1ps{jo}", name=f"l1ps{jo}")
                    ps1s.append(p1)
                nmm = 0
                for ji in range(4):
                    for dy in range(4):
                        for dx in range(4):
                            slab = l1w.tile([128, 512], f32, tag="w1slab")
                            nc.sync.dma_start(
                                out=slab[:, :],
                                in_=w1t[dy, dx, 128 * ji : 128 * (ji + 1), :])
                            rhs = pads1[ji][:, :].rearrange(
                                "c (h w) -> c h w", h=13)[:, dy : dy + 10, dx : dx + 10]
                            for jo in range(4):
                                nc.tensor.matmul(
                                    ps1s[jo][:, :],
                                    slab[:, 128 * jo : 128 * (jo + 1)], rhs,
                                    start=(nmm == 0), stop=(nmm == 63))
                            nmm += 1
                pads2 = []
                for jo in range(4):
                    raw = l1o.tile([128, 100], f32, tag=f"raw1_{jo}")
                    nc.vector.tensor_copy(raw[:, :], ps1s[jo][:, :])
                    relu = l1o.tile([128, 100], f32, tag=f"relu1_{jo}")
                    bn_relu(raw[:, :], 100, 128, 1, jo, relu[:, :])
                    pad = l1o.tile([128, 23 * 23], f32, tag=f"pad2_{jo}")
                    nc.vector.memset(pad[:, :], 0.0)
                    pv = pad[:, :].rearrange("c (h w) -> c h w", h=23)
                    rv = relu[:, :].rearrange("c (h w) -> c h w", h=10)
                    for a in range(2):
                        for b in range(2):
                            nc.vector.tensor_copy(
                                pv[:, a + 1 : a + 21 : 2, b + 1 : b + 21 : 2], rv[:, :, :])
                    pads2.append(pad)

                if _lvl >= 2:
                  # ---- L2: 512x20x20 conv 512->256 ----
                  with (
                      tc.tile_pool(name="l2_w", bufs=2) as l2w,
                      tc.tile_pool(name="l2_out", bufs=1) as l2o,
                  ):
                      psA = bps.tile([128, 400], f32, tag="cpsA")
                      psB = bps.tile([128, 400], f32, tag="cpsB")
                      nmm = 0
                      for ji in range(4):
                          for dy in range(4):
                              for dx in range(4):
                                  slab = l2w.tile([128, 256], f32, tag="w2slab")
                                  nc.sync.dma_start(
                                      out=slab[:, :],
                                      in_=w2t[dy, dx, 128 * ji : 128 * (ji + 1), :])
                                  rhs = pads2[ji][:, :].rearrange(
                                      "c (h w) -> c h w", h=23)[:, dy : dy + 20, dx : dx + 20]
                                  nc.tensor.matmul(
                                      psA[:, :], slab[:, 0:128], rhs,
                                      start=(nmm == 0), stop=(nmm == 63))
                                  nc.tensor.matmul(
                                      psB[:, :], slab[:, 128:256], rhs,
                                      start=(nmm == 0), stop=(nmm == 63))
                                  nmm += 1
                      pads3 = []
                      for jo, ps in enumerate((psA, psB)):
                          raw = l2o.tile([128, 400], f32, tag=f"raw2_{jo}")
                          nc.vector.tensor_copy(raw[:, :], ps[:, :])
                          relu = l2o.tile([128, 400], f32, tag=f"relu2_{jo}")
                          bn_relu(raw[:, :], 400, 128, 2, jo, relu[:, :])
                          pad = l2o.tile([128, 43 * 43], f32, tag=f"pad3_{jo}")
                          nc.vector.memset(pad[:, :], 0.0)
                          pv = pad[:, :].rearrange("c (h w) -> c h w", h=43)
                          rv = relu[:, :].rearrange("c (h w) -> c h w", h=20)
                          for a in range(2):
                              for b in range(2):
                                  nc.vector.tensor_copy(
                                      pv[:, a + 1 : a + 41 : 2, b + 1 : b + 41 : 2],
                                      rv[:, :, :])
                          pads3.append(pad)

                      if _lvl >= 3:
                        # ---- L3: 256x40x40 conv 256->128 ----
                        with (
                            tc.tile_pool(name="l3_w", bufs=1) as l3w,
                            tc.tile_pool(name="l3_out", bufs=1) as l3o,
                        ):
                            wsl3 = l3w.tile([128, 32 * 128], f32)
                            for ji in range(2):
                                for dy in range(4):
                                    for dx in range(4):
                                        si = (ji * 16 + dy * 4 + dx) * 128
                                        nc.sync.dma_start(
                                            out=wsl3[:, si : si + 128],
                                            in_=w3t[dy, dx, 128 * ji : 128 * (ji + 1), :])
                            raw3 = l3o.tile([128, 1600], f32)
                            for st in range(4):
                                ps = bps.tile([128, 400], f32, tag="cps", bufs=2)
                                nmm = 0
                                for ji in range(2):
                                    for dy in range(4):
                                        for dx in range(4):
                                            si = (ji * 16 + dy * 4 + dx) * 128
                                            rhs = pads3[ji][:, :].rearrange(
                                                "c (h w) -> c h w", h=43)[
                                                :, st * 10 + dy : st * 10 + dy + 10,
                                                dx : dx + 40]
                                            nc.tensor.matmul(
                                                ps[:, :], wsl3[:, si : si + 128], rhs,
                                                start=(nmm == 0), stop=(nmm == 31))
                                            nmm += 1
                                nc.vector.tensor_copy(
                                    raw3[:, 400 * st : 400 * (st + 1)], ps[:, :])
                            relu3 = l3o.tile([128, 1600], f32)
                            bn_relu(raw3[:, :], 1600, 128, 3, 0, relu3[:, :])
                            pad4 = l3o.tile([128, 83 * 83], f32)
                            nc.vector.memset(pad4[:, :], 0.0)
                            pv = pad4[:, :].rearrange("c (h w) -> c h w", h=83)
                            rv = relu3[:, :].rearrange("c (h w) -> c h w", h=40)
                            for a in range(2):
                                for b in range(2):
                                    nc.vector.tensor_copy(
                                        pv[:, a + 1 : a + 81 : 2, b + 1 : b + 81 : 2],
                                        rv[:, :, :])

                            if _lvl >= 4:
                              # ---- L4: 128x80x80 conv 128->64 ----
                              with (
                                  tc.tile_pool(name="l4_w", bufs=1) as l4w,
                                  tc.tile_pool(name="l4_out", bufs=1) as l4o,
                              ):
                                  wsl4 = l4w.tile([128, 16 * 64], f32)
                                  for dy in range(4):
                                      for dx in range(4):
                                          si = (dy * 4 + dx) * 64
                                          nc.sync.dma_start(
                                              out=wsl4[:, si : si + 64],
                                              in_=w4t[dy, dx, :, :])
                                  raw4 = l4o.tile([64, 6400], f32)
                                  for st in range(16):
                                      ps = bps.tile([64, 400], f32, tag="cps", bufs=2)
                                      nmm = 0
                                      for dy in range(4):
                                          for dx in range(4):
                                              si = (dy * 4 + dx) * 64
                                              rhs = pad4[:, :].rearrange(
                                                  "c (h w) -> c h w", h=83)[
                                                  :, st * 5 + dy : st * 5 + dy + 5,
                                                  dx : dx + 80]
                                              nc.tensor.matmul(
                                                  ps[:, :], wsl4[:, si : si + 64], rhs,
                                                  start=(nmm == 0), stop=(nmm == 15))
                                              nmm += 1
                                      nc.vector.tensor_copy(
                                          raw4[:, 400 * st : 400 * (st + 1)], ps[:, :])
                                  pad5 = l4o.tile([64, 83 * 83], f32)
                                  nc.vector.memset(pad5[:, :], 0.0)
                                  pv5 = pad5[:, :].rearrange("c (h w) -> c h w", h=83)[
                                      :, 1:81, 1:81]
                                  bn_relu(raw4[:, :], 6400, 64, 4, 0, pv5)

                                  if _lvl >= 5:
                                    # ---- L5: 64x80x80 conv 64->1 + tanh -> c ----
                                    with (
                                        tc.tile_pool(name="l5_w", bufs=1) as l5w,
                                        tc.tile_pool(name="l5_out", bufs=1) as l5o,
                                    ):
                                        wsl5 = l5w.tile([64, 16 * 32], f32)
                                        for dy in range(4):
                                            for dx in range(4):
                                                _p5 = (dy * 4 + dx) * 32
                                                nc.sync.dma_start(
                                                    out=wsl5[:, _p5 : _p5 + 32],
                                                    in_=w5t[dy, dx, :, :])
                                        for st in range(16):
                                            ps = bps.tile([32, 400], f32, tag="cps", bufs=2)
                                            nmm = 0
                                            for dy in range(4):
                                                for dx in range(4):
                                                    rhs = pad5[:, :].rearrange(
                                                        "c (h w) -> c h w", h=83)[
                                                        :, st * 5 + dy : st * 5 + dy + 5,
                                                        dx : dx + 80]
                                                    _p5 = (dy * 4 + dx) * 32
                                                    nc.tensor.matmul(
                                                        ps[:, :],
                                                        wsl5[:, _p5 : _p5 + 32],
                                                        rhs,
                                                        start=(nmm == 0), stop=(nmm == 15))
                                                    nmm += 1
                                            c32 = l5o.tile([32, 400], f32, tag="c32", name=f"c32_{st}")
                                            nc.scalar.activation(c32[:, :], ps[:, :], AF.Tanh)
                                            nc.sync.dma_start(
                                                out=c_scr[:, 400 * st : 400 * (st + 1)], in_=c32[:, :])

        # ================= Phase C: w = W_d2 @ c + b_d2 (sharded) ==========
        _skip_c = False
        if not _skip_c:
          with (
              tc.tile_pool(name="c_const", bufs=1) as ccp,
              tc.tile_pool(name="c_slab", bufs=2) as csp,
              tc.tile_pool(name="c_ps", bufs=1, space="PSUM") as cps,
          ):
              c_cols = ccp.tile([128, 50], f32)
              nc.sync.dma_start(
                  out=c_cols[:, :], in_=c_scr[0, :].rearrange("(f p) -> p f", p=128))
              bdc = ccp.tile([128, 5], f32)
              nc.sync.dma_start(out=bdc[:, :], in_=bd2_c[:, :])
              wtiles = {}
              for j in range(5):
                  wt_ps = cps.tile([128, 1], f32, tag=f"wps{j}", name=f"wps{j}")
                  wtiles[j] = wt_ps
              for k in range(50):
                  slab = csp.tile([128, MROWS_C], f32, tag="cslab")
                  nc.sync.dma_start(
                      out=slab[:, :], in_=wd2_t[128 * k : 128 * (k + 1), :])
                  for j in range(5):
                      cj = 128 if j < 4 else 84
                      nc.tensor.matmul(
                          wtiles[j][:cj, :], slab[:, 128 * j : 128 * j + cj],
                          c_cols[:, k : k + 1], start=(k == 0), stop=(k == 49))
              wdc = ccp.tile([128, 5], f32)
              for j in range(5):
                  cj = 128 if j < 4 else 84
                  nc.vector.tensor_tensor(
                      out=wdc[:cj, j : j + 1], in0=wtiles[j][:cj, :],
                      in1=bdc[:cj, j : j + 1], op=OP.add)
              for j in range(5):
                  cj = 128 if j < 4 else 84
                  nc.sync.dma_start(
                      out=wd_shard[128 * j : 128 * j + cj], in_=wdc[:cj, j])
        if not _skip_c:
            nc.gpsimd.collective_compute(
                "AllGather", OP.bypass, replica_groups=[list(range(NCORES))],
                ins=[wd_shard[:]], outs=[w_full[:]])

      if not with_scan:
          with tc.tile_pool(name="wout", bufs=1) as wop:
              w_sb0 = wop.tile([N, N], f32)
              nc.sync.dma_start(
                  out=w_sb0[:, :],
                  in_=w_full[0 : N * N].rearrange("(j i) -> j i", i=N))
              nc.sync.dma_start(out=w_out[:, :], in_=w_sb0[:, :])

      # ================= Phase D: spiking scan =========================
      # Column k of sblk holds s_{t0+k+1} (= output row t0+k); column k of
      # ublk holds u_{t0+k+1}.  Steady-state iteration k (t = t0 + k):
      #   DVE : sblk[k] = u_t - s_t            (output, off critical path)
      #   PE  : y_ps  = (-w).s_t  [start]      (s_t is a full cycle old)
      #         y_ps += ( w).u_t  [stop]       (u_t is the critical input)
      #   Act : ublk[k] = tanh(y_ps) = u_{t+1}
      if with_scan:
        # transpose chunking of one SCAN_B block (chunks of <=128 columns)
        chunks = []
        c0 = 0
        while c0 < SCAN_B:
            cw = min(128, SCAN_B - c0)
            chunks.append((c0, cw))
            c0 += cw
        with (
            tc.tile_pool(name="d_const", bufs=1) as dcp,
            tc.tile_pool(name="d_state", bufs=1) as dsp,
            tc.tile_pool(name="d_blk", bufs=3) as dbp,
            tc.tile_pool(name="d_out", bufs=2 * len(chunks)) as dop,
            tc.tile_pool(name="d_ps", bufs=4, space="PSUM") as dps,
            tc.tile_pool(name="t_ps", bufs=4, space="PSUM") as tpp,
        ):
            w_sb = dcp.tile([N, N], f32)
            nc.sync.dma_start(
                out=w_sb[:, :],
                in_=w_full[0 : N * N].rearrange("(j i) -> j i", i=N))
            wneg_sb = dcp.tile([N, N], f32)
            nc.vector.tensor_scalar_mul(wneg_sb[:, :], w_sb[:, :], -1.0)
            ident = dcp.tile([128, 128], f32)
            nc.sync.dma_start(out=ident[:, :], in_=ident_in[:, :])
            s_col = dsp.tile([N, 1], f32)
            nc.sync.dma_start(out=s_col[:, :], in_=s0_in[:, :])
            u_col = dsp.tile([N, 1], f32)

            # prime: u_0 = tanh(w . s_0)
            y0 = dps.tile([N, 1], f32, tag="y")
            nc.tensor.matmul(
                y0[:, :], w_sb[:, :], s_col[:, :], start=True, stop=True)
            nc.scalar.activation(u_col[:, :], y0[:, :], AF.Tanh)

            with tc.For_i(
                0, T, SCAN_B,
                hint_engines=(
                    mybir.EngineType.PE, mybir.EngineType.Activation,
                    mybir.EngineType.DVE),
            ) as iv:
                sblk = dbp.tile([N, SCAN_B], f32, tag="sblk")
                ublk = dbp.tile([N, SCAN_B], f32, tag="ublk")
                # chunk end column -> (c0, cw); transpose+DMA of a finished
                # chunk is interleaved into the loop (PE is idle mid-cycle)
                chunk_at = {c0 + cw - 1: (c0, cw) for c0, cw in chunks}

                def emit_chunk_out(c0, cw):
                    # PSUM->SBUF staging on DVE (walrus forbids GPSIMD<->PSUM;
                    # the DVE sub stream has a full cycle of slack per step)
                    tps = tpp.tile([128, N], f32, tag="tps")
                    nc.tensor.transpose(
                        tps[:cw, :], sblk[:, c0 : c0 + cw], ident[:N, :N])
                    tsb = dop.tile([128, N], f32, tag=f"tsb{c0}")
                    nc.vector.tensor_copy(tsb[:cw, :], tps[:cw, :])
                    nc.sync.dma_start(
                        out=out_traj[bass.ds(iv + c0, cw), :],
                        in_=tsb[:cw, :])

                for k in range(SCAN_B):
                    s_prev = s_col[:, :] if k == 0 else sblk[:, k - 1 : k]
                    u_prev = u_col[:, :] if k == 0 else ublk[:, k - 1 : k]
                    # output row t: s_{t+1} = u_t - s_t  (DVE)
                    nc.vector.tensor_tensor(
                        out=sblk[:, k : k + 1], in0=u_prev, in1=s_prev,
                        op=OP.subtract)
                    # y_{t+1} = w.u_t - w.s_t  (PE; mm_s first, it is old)
                    y_ps = dps.tile([N, 1], f32, tag="y")
                    nc.tensor.matmul(
                        y_ps[:, :], wneg_sb[:, :], s_prev,
                        start=True, stop=False)
                    nc.tensor.matmul(
                        y_ps[:, :], w_sb[:, :], u_prev,
                        start=False, stop=True)
                    # u_{t+1} = tanh(y_{t+1})  (Act)
                    nc.scalar.activation(
                        ublk[:, k : k + 1], y_ps[:, :], AF.Tanh)
                    # chunk c0..k fully written by sub(k) one cycle from now;
                    # emit it on the NEXT iteration for a settled RAW dep
                    if k - 1 in chunk_at:
                        emit_chunk_out(*chunk_at[k - 1])
                # carry state into the next block (same-engine, in-order)
                nc.scalar.copy(u_col[:, :], ublk[:, SCAN_B - 1 : SCAN_B])
                nc.vector.tensor_copy(
                    s_col[:, :], sblk[:, SCAN_B - 1 : SCAN_B])
                # last chunk finishes with the block
                emit_chunk_out(*chunk_at[SCAN_B - 1])

    return nc


def _marshal_inputs(inputs):
    """Build the 8 per-core input maps from the full problem inputs."""
    x = np.asarray(inputs["x"], np.float32).reshape(2048)
    win = np.asarray(inputs["W_in"], np.float32)
    b_in = np.asarray(inputs["b_in"], np.float32)
    wd2 = np.asarray(inputs["W_d2"], np.float32)
    bd2 = np.asarray(inputs["b_d2"], np.float32)
    sp = np.asarray(inputs["start_part"], np.float32)

    x_cols = np.ascontiguousarray(x.reshape(16, 128).T)
    g_all = np.zeros((128, 8), np.float32)
    be_all = np.zeros((128, 8), np.float32)
    g_all[:, 0:4] = _col_major_pad(np.asarray(inputs["g1"], np.float32), 4)
    g_all[:, 4:6] = _col_major_pad(np.asarray(inputs["g2"], np.float32), 2)
    g_all[:, 6:7] = _col_major_pad(np.asarray(inputs["g3"], np.float32), 1)
    g_all[:, 7:8] = _col_major_pad(np.asarray(inputs["g4"], np.float32), 1)
    be_all[:, 0:4] = _col_major_pad(np.asarray(inputs["be1"], np.float32), 4)
    be_all[:, 4:6] = _col_major_pad(np.asarray(inputs["be2"], np.float32), 2)
    be_all[:, 6:7] = _col_major_pad(np.asarray(inputs["be3"], np.float32), 1)
    be_all[:, 7:8] = _col_major_pad(np.asarray(inputs["be4"], np.float32), 1)
    wts = {
        "w1t": np.ascontiguousarray(
            np.asarray(inputs["w1"], np.float32).transpose(2, 3, 1, 0)),
        "w2t": np.ascontiguousarray(
            np.asarray(inputs["w2"], np.float32).transpose(2, 3, 1, 0)),
        "w3t": np.ascontiguousarray(
            np.asarray(inputs["w3"], np.float32).transpose(2, 3, 1, 0)),
        "w4t": np.ascontiguousarray(
            np.asarray(inputs["w4"], np.float32).transpose(2, 3, 1, 0)),
        "w5t": _pad_w5(np.asarray(inputs["w5"], np.float32)),
    }
    s0 = np.ascontiguousarray(sp[-1].reshape(N, 1))
    ident = np.eye(128, dtype=np.float32)

    wd2_pad = np.zeros((NCORES * MROWS_C, 6400), np.float32)
    wd2_pad[: wd2.shape[0]] = wd2
    bd2_pad = np.zeros(NCORES * MROWS_C, np.float32)
    bd2_pad[: bd2.shape[0]] = bd2

    in_maps = []
    for c in range(NCORES):
        m = {
            "x_cols": x_cols,
            "win_t": np.ascontiguousarray(
                win[MROWS_A * c : MROWS_A * (c + 1)].T),
            "bin_c": _col_major_pad(b_in[MROWS_A * c : MROWS_A * (c + 1)], 13),
            "g_all": g_all,
            "be_all": be_all,
            "wd2_t": np.ascontiguousarray(
                wd2_pad[MROWS_C * c : MROWS_C * (c + 1)].T),
            "bd2_c": _col_major_pad(bd2_pad[MROWS_C * c : MROWS_C * (c + 1)], 5),
            "s0": s0,
            "ident": ident,
        }
        m.update(wts)
        in_maps.append(m)
    return in_maps


LAST_EXEC_NS = None


def kernel(**inputs) -> np.ndarray:
    global LAST_EXEC_NS
    import os

    trace = bool(os.environ.get("KERNEL_TRACE"))
    nc = build_program(T_FULL)
    _reorder_waits(nc)
    _split_excess_waits(nc)
    in_maps = _marshal_inputs(inputs)
    res = run_bass_kernel_spmd(nc, in_maps, list(range(NCORES)), trace=trace)
    if res.exec_time_ns is not None:
        LAST_EXEC_NS = res.exec_time_ns
    out = np.asarray(res.results[0]["out"], np.float32)
    return out.reshape(1, T_FULL, N)


if __name__ == "__main__":
    # CoreSim selftest with a short scan (no hardware needed).
    import sys
    import time

    T_test = SCAN_B * 2
    nc = build_program(T_test)
    print("program built", flush=True)

    sys.path.insert(0, "/root/problem")
    import jax
    jax.config.update("jax_platform_name", "cpu")
    import reference

    inputs = reference.setup_inputs()
    inputs = {k: np.asarray(v) for k, v in inputs.items()}
    in_maps = _marshal_inputs(inputs)

    from concourse.bass_interp import MultiCoreSim

    t0 = time.time()
    sim = MultiCoreSim(nc, NCORES)
    for i in range(NCORES):
        for k, v in in_maps[i].items():
            sim.cores[i].tensor(k)[:] = v
    sim.simulate()
    print("sim time", time.time() - t0, flush=True)
    got = np.array(sim.cores[0].tensor("out"))

    # host reference for the short horizon
    w = np.load("/tmp/w.npy")
    s = np.asarray(inputs["start_part"])[-1].astype(np.float32)
    ref = np.empty((T_test, N), np.float32)
    for t in range(T_test):
        s = (np.tanh((s @ w).astype(np.float32)).astype(np.float32) - s).astype(
            np.float32)
        ref[t] = s
    err = np.abs(got - ref)
    rel = np.abs(got - ref) / (np.abs(ref) + 1e-6)
    print("traj absmax err:", err.max(), "rel max:", rel.max())
    print("first rows got:", got[0, :4], "ref:", ref[0, :4])

